# revision 41
# baseline (speedup 1.0000x reference)
"""Trainium2 Bass kernel for nn_LlamaEmbeddingClassifier.

Model: 2-layer Llama (D=512, 8 heads x 64, HID=1408, RoPE, RMSNorm) scoring
B=4 prompts against NLAB=5 label continuations (LBL=4 tokens) with an
lm_head over V=128000.

Strategy (8 NeuronCores, single SPMD launch):
  - Packed 528-token sequence [508 prefix | 5 x 4-token suffixes] with a
    custom attention mask; core c handles batch row (c % 4); cores 4-7
    duplicate 0-3 (SPMD program is uniform).  Layer 2 runs full K/V but a
    thin 16-query path for attention/MLP.
  - All weight matmuls (wq/wk/wv/wo/w1/w3/w2) run in fp8-e4m3 DoubleRow
    perf mode (2x PE throughput, half the weight HBM traffic).  The
    residual stream is stored scaled x64 so host-prescaled fp8 weights
    (x64) need no on-device compensation: RMSNorm is scale-invariant
    (eps folded as 64^2*eps), and both wo/w2 outputs land back on the
    x64 scale.  RoPE is applied via dual projections (W and W*P^T both
    in the fp8 weight blob) - no on-device transpose/permute matmul.
  - Attention probabilities are stored fp8; PV runs DoubleRow over
    key-tile pairs.  Softmax denominators come free via a ones-column in
    the value tile; exp is un-normalized (scores bounded ~|1.5|).
  - Per-layer weights are packed into one fp8 DRAM blob -> one big DMA
    per layer (512B+ contiguous runs, minimal HWDGE occupancy).  The
    fp8 lm_head shard (vocab/8 per core, padded to 16384 cols so chunk
    runs are exactly 512B) is fully resident in SBUF, prefetched during
    the transformer; the host subtracts the known exp(0) pad
    contribution from the sumexp.
  - The 16 final hidden rows per batch are AllGathered (tiny) so every
    core scores all 80 rows over its vocab shard.  Label-token logits
    come from a bf16 side matmul (lmsel) to keep the accuracy-critical
    path out of fp8.  Host combines: logsumexp across shards,
    lp = sel_logit - lse, summed per (batch,label).
"""

import math
import os
import sys
from contextlib import ExitStack

for _p in ("/opt/trn_rl_repo", "/root/.axon_site/_ro/trn_rl_repo"):
    if os.path.isdir(_p) and _p not in sys.path:
        sys.path.insert(0, _p)

import ml_dtypes
import numpy as np

import concourse.bass as bass
import concourse.tile as tile
from concourse import bacc, mybir
from concourse.bass_utils import run_bass_kernel_spmd

BF16 = ml_dtypes.bfloat16
FP8 = np.dtype(ml_dtypes.float8_e4m3)

# Problem dims (hardcoded per contract)
V, D, NH, NL, HID = 128000, 512, 8, 2, 1408
HD, HALF = 64, 32
B, T, NLAB, LBL = 4, 508, 5, 4
EPS = 1e-5
NCORES = 8
SUF = NLAB * LBL            # 20 suffix tokens
NTOK = T + SUF              # 528 packed tokens
KT = D // 128               # 4 K-tiles over D
TT = (NTOK + 127) // 128    # 5 token tiles (last has 16 rows)
TP = 3                      # key-tile pairs for DoubleRow PV
HT = HID // 128             # 11 tiles over HID
HA = 6                      # DoubleRow passes over padded HID (1536)
NROW = B * NLAB * LBL       # 80 scoring rows
NSEL = NLAB * LBL           # 20 selected lm_head columns
NQ = 1 + NLAB * (LBL - 1)   # 16 thin-path query positions
VSH = V // NCORES           # 16000 vocab shard per core
VCH = 512                   # vocab chunk (512B fp8 runs, 2KB psum bank)
NVCH = 32                   # chunks over the padded shard
VPAD = NVCH * VCH - VSH     # 384 zero-padded columns -> exp(0)=1 each
QC = 264                    # q chunk (2 chunks of 264 = 528)

RS = 64.0                   # residual-stream scale (h stores 64*h_true)
W_SC = 64.0                 # host scale for wq/wk/wv/wo/w1/w3 (fp8 range)
W2_SC = 16.0                # host scale for w2
G1_SC = 4.0                 # on-device scale of stored g1 (W2_SC*G1_SC=RS)
EPS_S = EPS * RS * RS       # rms eps on the scaled residual
LMH_SCALE = 32.0            # host premultiply of fp8 lm_head
HS_SCALE = 4.0              # device premultiply of fp8 hs copies

# causal block structure: q-chunk 0 (q<264) only sees k-tiles 0,1; q-chunk 1
# sees all.  (Queries 256..263 lose keys 256..263 - tiny, within tolerance.)
CH_MTS = {0: (0, 1), 1: (0, 1, 2, 3, 4)}
CH_TPS = {0: (0,), 1: (0, 1, 2)}   # same structure as key-tile pairs
# exp only over columns that are not fully causally masked
EXPR = {(0, 0): (0, QC), (1, 0): (128, QC),
        (0, 1): (QC, NTOK), (1, 1): (QC, NTOK), (2, 1): (QC, NTOK),
        (3, 1): (384, NTOK), (4, 1): (512, NTOK)}
# mask-multiply ranges (within the exp'd region) + offset into packed maskc
MSEG = {0: (0, 128, 0), 1: (128, 256, 128), 2: (QC, 384, 256),
        3: (384, NTOK, 376), 4: (512, NTOK, 520)}
MSEG_SZ = 536

# fp8 weight-blob element offsets (per partition), layer-major
WB_K, WB_KR, WB_Q, WB_QR = 0, 2048, 4096, 6144
WB_V, WB_O = 8192, 10240
WB_W1, WB_W3, WB_W2 = 12288, 17920, 23552
WB_SZ = 29696
# consts blob (bf16) element offsets
CB_C, CB_S = 0, NTOK
CB_CQ, CB_SQ = 2 * NTOK, 2 * NTOK + NQ
CB_MD = 2 * NTOK + 2 * NQ                 # packed trimmed mask segments
CB_MQ = CB_MD + MSEG_SZ                   # [5, 32] head-pair thin mask
CB_SEL = CB_MQ + TT * 2 * NQ              # [4, 20] lmsel
CB_SZ = CB_SEL + KT * NSEL

_CACHE = {}


def _tok_rows(tt):
    return min(128, NTOK - tt * 128)


def build_nc(use_collective=True):
    nc = bacc.Bacc("TRN2", num_devices=NCORES)
    f32, bf16, fp8 = mybir.dt.float32, mybir.dt.bfloat16, mybir.dt.float8e4
    DR = mybir.MatmulPerfMode.DoubleRow

    # ---- I/O ----
    x0T = nc.dram_tensor("x0T", [128, KT, NTOK], bf16,
                         kind="ExternalInput")
    wb = nc.dram_tensor("wb", [NL, 128, WB_SZ], fp8, kind="ExternalInput")
    cb = nc.dram_tensor("cb", [128, CB_SZ], bf16, kind="ExternalInput")
    lmh = nc.dram_tensor("lmh", [128, NVCH, 2, 2, VCH], fp8,
                         kind="ExternalInput")

    se_out = nc.dram_tensor("se_out", [NROW, NVCH], bf16,
                            kind="ExternalOutput")
    sel_out = nc.dram_tensor("sel_out", [NROW, NSEL], f32,
                             kind="ExternalOutput")

    with tile.TileContext(nc) as tc, ExitStack() as ctx:
        consts = ctx.enter_context(tc.tile_pool(name="consts", bufs=1))
        wpool = ctx.enter_context(tc.tile_pool(name="weights", bufs=1))
        lpool = ctx.enter_context(tc.tile_pool(name="lmh", bufs=1))
        acts = ctx.enter_context(tc.tile_pool(name="acts", bufs=1))
        scr = ctx.enter_context(tc.tile_pool(name="scratch", bufs=6))
        ppool = ctx.enter_context(tc.tile_pool(name="p", bufs=2))
        psum = ctx.enter_context(tc.tile_pool(name="psum", bufs=7,
                                              space="PSUM"))
        psmall = ctx.enter_context(tc.tile_pool(name="psmall", bufs=1,
                                                space="PSUM"))
        dram = ctx.enter_context(tc.tile_pool(name="dram", bufs=1,
                                              space="DRAM"))

        # ---- resident tiles ----
        h = acts.tile([128, KT, NTOK], bf16, name="h")
        cbs = consts.tile([128, CB_SZ], bf16, name="cbs")
        wsb = wpool.tile([128, NL, WB_SZ], fp8, name="wsb")
        lsb = lpool.tile([128, NVCH, 2, 2, VCH], fp8, name="lsb")

        # ---- input DMAs, in pipeline order (single shared DMA pipe) ----
        nc.sync.dma_start(out=h[:], in_=x0T[:])
        nc.sync.dma_start(out=cbs[:], in_=cb[:])
        nc.sync.dma_start(out=wsb[:, 0, :WB_V], in_=wb[0][:, :WB_V])
        nc.sync.dma_start(out=wsb[:, 0, WB_V:], in_=wb[0][:, WB_V:])
        nc.sync.dma_start(out=wsb[:, 1, :], in_=wb[1][:])
        nc.sync.dma_start(out=lsb[:, :NVCH // 2], in_=lmh[:, :NVCH // 2])
        nc.sync.dma_start(out=lsb[:, NVCH // 2:], in_=lmh[:, NVCH // 2:])

        # ---- const views ----
        C128 = cbs[:, CB_C:CB_C + NTOK]
        S128 = cbs[:, CB_S:CB_S + NTOK]
        Cq = cbs[:, CB_CQ:CB_CQ + NQ]
        Sq = cbs[:, CB_SQ:CB_SQ + NQ]
        maskc = cbs[:, CB_MD:CB_MD + MSEG_SZ]
        maskq = cbs[:, CB_MQ:CB_MQ + TT * 2 * NQ].rearrange(
            "p (t q) -> p t q", t=TT)
        lmsel_sb = cbs[:, CB_SEL:CB_SEL + KT * NSEL].rearrange(
            "p (k c) -> p k c", k=KT)

        def wv_(l, off, n, a=2):
            return wsb[:, l, off:off + a * 2 * n].rearrange(
                "p (a s n) -> p a s n", a=a, s=2)

        ones_col = consts.tile([128, 1], bf16)
        nc.vector.memset(ones_col, 1.0)
        eps_sb = consts.tile([1, 1], f32)
        nc.vector.memset(eps_sb, float(EPS_S))

        # activations
        xn8 = acts.tile([128, 2, 2, NTOK], fp8, name="xn8")
        xn8b = acts.tile([128, 2, 2, NTOK], fp8, name="xn8b")
        kTt = acts.tile([128, KT, NTOK], bf16, name="kTt")
        qT = acts.tile([128, KT, NTOK], bf16, name="qT")
        oT8 = acts.tile([128, 2, 2, NTOK], fp8, name="oT8")
        g1 = acts.tile([128, HA, 2, NTOK], fp8, name="g1")
        # v with interleaved ones column plus one pad column (so the
        # DoubleRow lhsT pair-stride is 528 = a multiple of 16 bytes), in
        # key-tile-pair layout: [keys(128), pair, slot, 8 x (64 v | one |pad)]
        HDP = HD + 2
        v_aug = acts.tile([128, TP, 2, NH * HDP], fp8, name="v_aug")
        v5 = v_aug.rearrange("p t s (h c) -> p t s h c", c=HDP)
        nc.gpsimd.memset(v5[:, :, :, :, HD:], 1.0)    # ones (+pad) columns
        nc.gpsimd.memset(v5[:, 2, 1, :, :HD], 0.0)    # pad slot (tile 5)
        nc.gpsimd.memset(v5[:, 2, 0, :, :HD], 0.0)    # tile-4 (rows 0:16 get
        # overwritten by the real v projection; tail rows stay zero)
        nc.gpsimd.memset(g1[:, HA - 1, 1, :], 0.0)    # padded HID slot

        hq = acts.tile([128, KT, NQ], bf16, name="hq")
        xnq8 = acts.tile([128, 2, 2, NQ], fp8, name="xnq8")
        xnq8b = acts.tile([128, 2, 2, NQ], fp8, name="xnq8b")
        qTq = acts.tile([128, KT, NQ], bf16, name="qTq")
        oTq8 = acts.tile([128, 2, 2, NQ], fp8, name="oTq8")
        g1q = acts.tile([128, HA, 2, NQ], fp8, name="g1q")
        nc.gpsimd.memset(g1q[:, HA - 1, 1, :], 0.0)

        FULL_CH = ((0, QC), (QC, NTOK))
        THIN_CH = ((0, NQ),)

        def rms(src, dest, n, chunks, sq_eng, mul_eng=None):
            mul_eng = mul_eng or nc.vector
            """dest (fp8, DR layout) = src * rsqrt(mean_D(src^2)+eps').

            Emitted per column-chunk so downstream projections of chunk 0
            can start while chunk 1 is still normalizing.
            """
            for c0, c1 in chunks:
                w = c1 - c0
                sq = scr.tile([128, KT, QC], bf16, name="sq", bufs=2)
                for kt in range(KT):
                    sq_eng.tensor_mul(out=sq[:, kt, :w],
                                      in0=src[:, kt, c0:c1],
                                      in1=src[:, kt, c0:c1])
                ss = psmall.tile([1, QC], f32, name="ss", tag="small")
                for kt in range(KT):
                    nc.tensor.matmul(ss[:, :w], ones_col[:], sq[:, kt, :w],
                                     start=(kt == 0), stop=(kt == KT - 1))
                # rsqrt(x) = exp(-0.5*ln(x)); Ln/Exp share an act table
                lnb = scr.tile([1, QC], f32, name="lnb")
                nc.scalar.activation(out=lnb[:, :w], in_=ss[:, :w],
                                     func=mybir.ActivationFunctionType.Ln,
                                     scale=1.0 / D, bias=eps_sb[:])
                rstd = scr.tile([1, QC], bf16, name="rstd")
                nc.scalar.activation(out=rstd[:, :w], in_=lnb[:, :w],
                                     func=mybir.ActivationFunctionType.Exp,
                                     scale=-0.5)
                rb = scr.tile([128, QC], bf16, name="rms_rb", bufs=2)
                nc.gpsimd.partition_broadcast(rb[:, :w], rstd[:, :w])
                for kt in range(KT):
                    mul_eng.tensor_mul(out=dest[:, kt // 2, kt % 2, c0:c1],
                                       in0=src[:, kt, c0:c1],
                                       in1=rb[:, :w])

        def proj_rope(specs, xn, chunks, ctb, stb, mts=None):
            """dest[Dout, n] (bf16) = rope((xn @ W).T) via two fp8 DR projs.

            specs: list of (dest, w_raw_view, w_rot_view); tiles of the
            different projections are interleaved for engine overlap.
            """
            for mt in (range(KT) if mts is None else mts):
                msl = slice(mt * 128, (mt + 1) * 128)
                for c0, c1 in chunks:
                    n = c1 - c0
                    for dest, wv_raw, wv_rot in specs:
                        psA = psum.tile([128, QC], f32, name="pjA", tag="mm")
                        psB = psum.tile([128, QC], f32, name="pjB", tag="mm")
                        for a in range(2):
                            nc.tensor.matmul(psA[:, :n], wv_raw[:, a, :, msl],
                                             xn[:, a, :, c0:c1],
                                             start=(a == 0), stop=(a == 1),
                                             perf_mode=DR)
                        for a in range(2):
                            nc.tensor.matmul(psB[:, :n], wv_rot[:, a, :, msl],
                                             xn[:, a, :, c0:c1],
                                             start=(a == 0), stop=(a == 1),
                                             perf_mode=DR)
                        t1 = scr.tile([128, QC], bf16, name="rope_t1")
                        nc.vector.tensor_mul(out=t1[:, :n], in0=psA[:, :n],
                                             in1=ctb[:, c0:c1])
                        t2 = scr.tile([128, QC], bf16, name="rope_t2")
                        nc.vector.tensor_mul(out=t2[:, :n], in0=psB[:, :n],
                                             in1=stb[:, c0:c1])
                        nc.gpsimd.tensor_add(out=dest[:, mt, c0:c1],
                                             in0=t1[:, :n], in1=t2[:, :n])

        def v_proj(wv_v, xn, n_tiles, ncols):
            """v_aug[:, tp, s, h, :HD] = (xn @ Wv).T / RS in pair layout."""
            for mt in range(n_tiles):
                mr = min(128, ncols - mt * 128)
                ps = psum.tile([128, D], f32, name="v_ps", tag="mm")
                if mr == 128:
                    for a in range(2):
                        nc.tensor.matmul(
                            ps[:mr, :], xn[:, a, :, mt * 128:mt * 128 + mr],
                            wv_v[:, a],
                            start=(a == 0), stop=(a == 1), perf_mode=DR)
                else:  # tail: plain fp8 (DR needs a full 128-col stationary)
                    for i, (a, s_) in enumerate(
                            [(a, s_) for a in range(2) for s_ in range(2)]):
                        nc.tensor.matmul(
                            ps[:mr, :], xn[:, a, s_, mt * 128:mt * 128 + mr],
                            wv_v[:, a, s_],
                            start=(i == 0), stop=(i == 3))
                nc.vector.tensor_scalar_mul(
                    out=v5[:mr, mt // 2, mt % 2, :, :HD],
                    in0=ps.rearrange("p (h c) -> p h c", c=HD)[:mr],
                    scalar1=1.0 / RS)

        def attn_norm(po, a, s, b, dest8, cs, n):
            """dest8 = po[:HD]/po[HD] (softmax denominator), fp8 out."""
            rs_t = scr.tile([1, QC], f32, name="attn_rs")
            nc.vector.reciprocal(out=rs_t[:, :n], in_=po[HD:HD + 1, :n])
            rb_t = scr.tile([64, QC], f32, name="attn_rb")
            nc.gpsimd.partition_broadcast(rb_t[:, :n], rs_t[:, :n])
            nc.vector.tensor_mul(
                out=dest8[64 * b:64 * b + 64, a, s, cs],
                in0=po[:HD, :n], in1=rb_t[:, :n])

        def attn_head_full(hh):
                tq = hh // 2
                rq = slice(64 * (hh % 2), 64 * (hh % 2) + 64)
                a, s, bb = hh // 4, (hh // 2) % 2, hh % 2
                p_sb = ppool.tile([128, TP, 2, NTOK], fp8, name="p_sb")
                if hh < 2:  # zero exp-trimmed regions in both buffers
                    nc.gpsimd.memset(p_sb[:, 0, 1, 0:128], 0.0)
                    nc.gpsimd.memset(p_sb[:, 1, 1, QC:384], 0.0)
                    nc.gpsimd.memset(p_sb[:, 2, 0, QC:], 0.0)
                    nc.gpsimd.memset(p_sb[:, 2, 1, QC:], 0.0)
                for mt in range(TT):
                    mr = _tok_rows(mt)
                    for ch in range(2):
                        if mt not in CH_MTS[ch]:
                            continue
                        e0, e1 = EXPR[(mt, ch)]
                        cs = slice(ch * QC, (ch + 1) * QC)
                        ps = psum.tile([128, QC], f32, name="score_ps",
                                       tag="mm")
                        nc.tensor.matmul(
                            ps[:mr, :e1 - e0],
                            kTt[rq, tq, mt * 128:mt * 128 + mr],
                            qT[rq, tq, e0:e1], start=True, stop=True)
                        nc.scalar.activation(
                            out=p_sb[:mr, mt // 2, mt % 2, e0:e1],
                            in_=ps[:mr, :e1 - e0],
                            func=mybir.ActivationFunctionType.Exp,
                            scale=1.0 / math.sqrt(HD))
                    m0, m1, mo = MSEG[mt]
                    nc.gpsimd.tensor_mul(
                        out=p_sb[:mr, mt // 2, mt % 2, m0:m1],
                        in0=p_sb[:mr, mt // 2, mt % 2, m0:m1],
                        in1=maskc[:mr, mo:mo + m1 - m0])
                for ch in range(2):
                    cs = slice(ch * QC, (ch + 1) * QC)
                    tps = CH_TPS[ch]
                    po = psum.tile([128, QC], f32, name="pv_ps", tag="mm")
                    for i, tp in enumerate(tps):
                        nc.tensor.matmul(
                            po[:HD + 2, :],
                            v_aug[:, tp, :, hh * HDP:(hh + 1) * HDP],
                            p_sb[:, tp, :, cs],
                            start=(i == 0), stop=(i == len(tps) - 1),
                            perf_mode=DR)
                    attn_norm(po, a, s, bb, oT8, cs, QC)

        def build_qz():
            # head-pair batching: kTt's 128 partitions hold two heads; the
            # query block is zero-padded so one matmul yields both heads'
            # scores side by side ([mr, 32], offset 0 - HW-safe).
            qz = scr.tile([128, KT, 2 * NQ], bf16, name="qz", bufs=1)
            nc.vector.memset(qz[:], 0.0)
            for tq in range(KT):
                nc.vector.tensor_copy(out=qz[0:64, tq, 0:NQ],
                                      in_=qTq[0:64, tq, :])
                nc.vector.tensor_copy(out=qz[64:128, tq, NQ:2 * NQ],
                                      in_=qTq[64:128, tq, :])
            return qz

        def attn_group_thin(tq, qz):
                p_sb = ppool.tile([128, TP, 2, 2 * NQ], fp8, name="pq_sb")
                if tq < 2:
                    nc.gpsimd.memset(p_sb[:, 2, 1, :], 0.0)
                    nc.gpsimd.memset(p_sb[:, 2, 0, :], 0.0)
                for mt in range(TT):
                    mr = _tok_rows(mt)
                    ps = psum.tile([128, QC], f32, name="score_ps", tag="mm")
                    nc.tensor.matmul(
                        ps[:mr, :2 * NQ],
                        kTt[:, tq, mt * 128:mt * 128 + mr],
                        qz[:, tq, :], start=True, stop=True)
                    nc.scalar.activation(
                        out=p_sb[:mr, mt // 2, mt % 2, :],
                        in_=ps[:mr, :2 * NQ],
                        func=mybir.ActivationFunctionType.Exp,
                        scale=1.0 / math.sqrt(HD))
                    nc.gpsimd.tensor_mul(
                        out=p_sb[:mr, mt // 2, mt % 2, :],
                        in0=p_sb[:mr, mt // 2, mt % 2, :],
                        in1=maskq[:mr, mt, :])
                for half in range(2):
                    hh = 2 * tq + half
                    a, s, bb = hh // 4, (hh // 2) % 2, hh % 2
                    nsl = slice(half * NQ, (half + 1) * NQ)
                    po = psum.tile([128, QC], f32, name="pv_ps", tag="mm")
                    for tp in range(TP):
                        nc.tensor.matmul(
                            po[:HD + 2, :NQ],
                            v_aug[:, tp, :, hh * HDP:(hh + 1) * HDP],
                            p_sb[:, tp, :, nsl],
                            start=(tp == 0), stop=(tp == TP - 1),
                            perf_mode=DR)
                    attn_norm(po, a, s, bb, oTq8, slice(0, NQ), NQ)

        def accum_proj_dr(w_view, npass, src8, dest, chunks, add_eng):
            """dest += (src8 DR-matmul w).T ; w_view [128, npass, 2, Dout]."""
            for mt in range(KT):
                msl = slice(mt * 128, (mt + 1) * 128)
                for c0, c1 in chunks:
                    n = c1 - c0
                    ps = psum.tile([128, QC], f32, name="acc_ps", tag="mm")
                    for a in range(npass):
                        nc.tensor.matmul(
                            ps[:, :n], w_view[:, a, :, msl],
                            src8[:, a, :, c0:c1],
                            start=(a == 0), stop=(a == npass - 1),
                            perf_mode=DR)
                    add_eng.tensor_add(out=dest[:, mt, c0:c1],
                                       in0=dest[:, mt, c0:c1], in1=ps[:, :n])

        def mlp(l, xn, gdest, chunks, dest, n_ht, add_eng):
            w1v, w3v = wv_(l, WB_W1, HID), wv_(l, WB_W3, HID)
            w2v = wv_(l, WB_W2, D, a=HA)
            for mt in range(n_ht):
                msl = slice(mt * 128, (mt + 1) * 128)
                for ci, (c0, c1) in enumerate(chunks):
                    n = c1 - c0
                    ps3 = psum.tile([128, QC], f32, name="g3_ps", tag="mm")
                    for a in range(2):
                        nc.tensor.matmul(ps3[:, :n], w3v[:, a, :, msl],
                                         xn[:, a, :, c0:c1],
                                         start=(a == 0), stop=(a == 1),
                                         perf_mode=DR)
                    ps1 = psum.tile([128, QC], f32, name="g1_ps", tag="mm")
                    for a in range(2):
                        nc.tensor.matmul(ps1[:, :n], w1v[:, a, :, msl],
                                         xn[:, a, :, c0:c1],
                                         start=(a == 0), stop=(a == 1),
                                         perf_mode=DR)
                    tsil = scr.tile([128, QC], bf16, name="tsil")
                    nc.scalar.activation(
                        out=tsil[:, :n], in_=ps1[:, :n],
                        func=mybir.ActivationFunctionType.Silu,
                        scale=1.0 / W_SC)
                    # g1 = (ps3 * G1_SC/W_SC) * silu  (fused; DVE - reads PSUM)
                    nc.vector.scalar_tensor_tensor(
                        out=gdest[:, mt // 2, mt % 2, c0:c1],
                        in0=ps3[:, :n], scalar=G1_SC / W_SC,
                        in1=tsil[:, :n],
                        op0=mybir.AluOpType.mult, op1=mybir.AluOpType.mult)
            accum_proj_dr(w2v, HA, gdest, dest, chunks, add_eng)

        def gather_q(dest, src):
            """cols: 0 <- 507; 1+3l+j <- 508+4l+j (j=0..2); [128, k, cols]"""
            for kt in range(src.shape[1]):
                nc.vector.tensor_copy(out=dest[:, kt, 0:1],
                                      in_=src[:, kt, T - 1:T])
                nc.vector.tensor_copy(
                    out=dest[:, kt, 1:NQ].rearrange("p (l s) -> p l s", s=3),
                    in_=src[:, kt, T:T + SUF].rearrange(
                        "p (l s) -> p l s", s=LBL)[:, :, 0:3])

        def gather_q4(dest, src):
            for a in range(2):
                for s_ in range(2):
                    nc.vector.tensor_copy(out=dest[:, a, s_, 0:1],
                                          in_=src[:, a, s_, T - 1:T])
                    nc.vector.tensor_copy(
                        out=dest[:, a, s_, 1:NQ].rearrange(
                            "p (l s) -> p l s", s=3),
                        in_=src[:, a, s_, T:T + SUF].rearrange(
                            "p (l s) -> p l s", s=LBL)[:, :, 0:3])

        # ================= transformer =================
        # Attention of heads (2t, 2t+1) only needs k/q tile t, so emit each
        # projection tile followed immediately by its two heads - the PE's
        # in-order queue + psum backpressure otherwise delays the first
        # score matmul until every projection tile has drained.
        for l in range(NL):
            full = l < NL - 1
            rms(h, xn8, NTOK, FULL_CH, nc.vector if full else nc.gpsimd,
                nc.vector if full else nc.gpsimd)
            if full:
                specs = [(kTt, wv_(l, WB_K, D), wv_(l, WB_KR, D)),
                         (qT, wv_(l, WB_Q, D), wv_(l, WB_QR, D))]
                proj_rope(specs, xn8, FULL_CH, C128, S128)
                v_proj(wv_(l, WB_V, D), xn8, TT, NTOK)
                for hh in range(NH):
                    attn_head_full(hh)
                accum_proj_dr(wv_(l, WB_O, D), 2, oT8, h, FULL_CH, nc.vector)
                rms(h, xn8b, NTOK, FULL_CH, nc.gpsimd, nc.gpsimd)
                mlp(l, xn8b, g1, FULL_CH, h, HT, nc.vector)
            else:
                gather_q(hq, h)
                gather_q4(xnq8, xn8)
                proj_rope([(qTq, wv_(l, WB_Q, D), wv_(l, WB_QR, D))],
                          xnq8, THIN_CH, Cq, Sq)
                qz = build_qz()
                kspec = [(kTt, wv_(l, WB_K, D), wv_(l, WB_KR, D))]
                proj_rope(kspec, xn8, FULL_CH, C128, S128)
                v_proj(wv_(l, WB_V, D), xn8, TT, NTOK)
                for tq in range(KT):
                    attn_group_thin(tq, qz)
                accum_proj_dr(wv_(l, WB_O, D), 2, oTq8, hq, THIN_CH,
                              nc.vector)
                rms(hq, xnq8b, NQ, THIN_CH, nc.vector)
                mlp(l, xnq8b, g1q, THIN_CH, hq, HT, nc.vector)

        # ============ final norm + extract + AllGather ============
        xnf = scr.tile([128, KT, NQ], bf16, name="xnf")
        # bf16 out (not fp8): feeds the accuracy-critical lmsel path
        sqf = scr.tile([128, KT, NQ], bf16, name="sqf", bufs=1)
        for kt in range(KT):
            nc.vector.tensor_mul(out=sqf[:, kt, :], in0=hq[:, kt, :],
                                 in1=hq[:, kt, :])
        ssf = psmall.tile([1, QC], f32, name="ss", tag="small")
        for kt in range(KT):
            nc.tensor.matmul(ssf[:, :NQ], ones_col[:], sqf[:, kt, :],
                             start=(kt == 0), stop=(kt == KT - 1))
        lnf = scr.tile([1, QC], f32, name="lnb")
        nc.scalar.activation(out=lnf[:, :NQ], in_=ssf[:, :NQ],
                             func=mybir.ActivationFunctionType.Ln,
                             scale=1.0 / D, bias=eps_sb[:])
        rstdf = scr.tile([1, NQ], bf16, name="rstdf")
        nc.scalar.activation(out=rstdf[:], in_=lnf[:, :NQ],
                             func=mybir.ActivationFunctionType.Exp,
                             scale=-0.5)
        rbf = scr.tile([128, NQ], bf16, name="rbf")
        nc.gpsimd.partition_broadcast(rbf[:], rstdf[:])
        for kt in range(KT):
            nc.vector.tensor_mul(out=xnf[:, kt, :], in0=hq[:, kt, :],
                                 in1=rbf[:])

        hsT_own = acts.tile([128, KT, NSEL], bf16, name="hsT_own")
        for kt in range(KT):
            for ll in range(NLAB):
                nc.vector.tensor_copy(
                    out=hsT_own[:, kt, ll * LBL:ll * LBL + 1],
                    in_=xnf[:, kt, 0:1])
            nc.vector.tensor_copy(
                out=hsT_own.rearrange("p k (l s) -> p k l s", s=LBL)[
                    :, kt, :, 1:LBL],
                in_=xnf[:, kt, 1:NQ].rearrange("p (l s) -> p l s", s=3))

        cc_in = dram.tile([D, NSEL], bf16)
        cc_out = dram.tile([NCORES * D, NSEL], bf16)
        nc.sync.dma_start(
            out=cc_in.rearrange("(k p) c -> p k c", p=128), in_=hsT_own[:])
        if use_collective:
            nc.gpsimd.collective_compute(
                "AllGather",
                mybir.AluOpType.bypass,
                replica_groups=[list(range(NCORES))],
                ins=[cc_in.opt()],
                outs=[cc_out.opt()],
            )
        else:  # timeline-sim variant: emulate with local copies, spread
            # across four queues so the copies run concurrently
            qs = [nc.sync, nc.scalar, nc.gpsimd]
            for r in range(NCORES):
                qs[r % 3].dma_start(
                    out=cc_out[r * D:(r + 1) * D, :], in_=cc_in[:])

        hsT_all = acts.tile([128, KT, B, NSEL], bf16, name="hsT_all")
        cc_view = cc_out.rearrange("(b k p) c -> p k b c", b=NCORES, p=128)
        qs2 = [nc.sync, nc.scalar, nc.gpsimd, nc.scalar]
        for kt in range(KT):
            qs2[kt].dma_start(out=hsT_all[:, kt], in_=cc_view[:, kt, 0:B, :])

        # ================= lm_head phase =================
        hs8 = acts.tile([128, 2, 2, B * NSEL], fp8, name="hs8")
        for kt in range(KT):
            nc.vector.tensor_scalar_mul(
                out=hs8[:, kt // 2, kt % 2, :],
                in0=hsT_all.rearrange("p k b c -> p k (b c)")[:, kt, :],
                scalar1=HS_SCALE)
        se_sb = acts.tile([NROW, NVCH], bf16, name="se_sb")
        escale = 1.0 / (LMH_SCALE * HS_SCALE)
        for j in range(NVCH):
            pl = psum.tile([NROW, VCH], f32, name="lm_ps", tag="mm")
            for a in range(2):
                nc.tensor.matmul(pl[:], hs8[:, a], lsb[:, j, a],
                                 start=(a == 0), stop=(a == 1),
                                 perf_mode=DR)
            esc = scr.tile([NROW, VCH], bf16, name="esc", bufs=3)
            nc.scalar.activation(
                out=esc[:], in_=pl[:],
                func=mybir.ActivationFunctionType.Exp,
                scale=escale)
            # row-sum on the (otherwise idle) DVE instead of the scalar
            # engine's accumulator read
            with nc.allow_low_precision("bf16 partial sumexp: one rounding "
                                        "per 512-col chunk, ~0.07% on lse"):
                nc.vector.tensor_reduce(out=se_sb[:, j:j + 1], in_=esc[:],
                                        axis=mybir.AxisListType.X,
                                        op=mybir.AluOpType.add)
        nc.sync.dma_start(out=se_out[:], in_=se_sb[:])

        psel = psmall.tile([NROW, NSEL], f32, name="sel_ps", tag="small")
        for kt in range(KT):
            nc.tensor.matmul(
                psel[:],
                hsT_all.rearrange("p k b c -> p k (b c)")[:, kt, :],
                lmsel_sb[:, kt, :],
                start=(kt == 0), stop=(kt == KT - 1))
        sel_sb = scr.tile([NROW, NSEL], f32, name="sel_sb")
        nc.scalar.copy(out=sel_sb[:], in_=psel[:])
        nc.sync.dma_start(out=sel_out[:], in_=sel_sb[:])

    nc.finalize()
    return nc


def _get_nc():
    if "nc" not in _CACHE:
        _CACHE["nc"] = build_nc()
    return _CACHE["nc"]


def _build_masks():
    """full mask [TT,128,NTOK] and thin mask [TT,128,NQ] over (k, q)."""
    k_idx = np.arange(TT * 128)
    kpos = np.where(k_idx < T, k_idx, 0)
    klab = np.where(k_idx < T, -1, (k_idx - T) // LBL)
    koff = np.where(k_idx < T, 0, (k_idx - T) % LBL)
    kvalid = k_idx < NTOK

    def allow(qpos, qlab, qoff):
        kp = kpos[:, None]; kl = klab[:, None]; ko = koff[:, None]
        prefix_k = kl == -1
        prefix_q = (qlab == -1)[None, :]
        a = np.where(
            prefix_q,
            prefix_k & (kp <= qpos[None, :]),
            prefix_k | ((kl == qlab[None, :]) & (ko <= qoff[None, :])),
        )
        return (a & kvalid[:, None]).astype(np.float32)

    q_idx = np.arange(NTOK)
    qpos = np.where(q_idx < T, q_idx, 0)
    qlab = np.where(q_idx < T, -1, (q_idx - T) // LBL)
    qoff = np.where(q_idx < T, 0, (q_idx - T) % LBL)
    maskd = allow(qpos, qlab, qoff).reshape(TT, 128, NTOK)

    tq = np.array([T - 1] + [T + 4 * l + j for l in range(NLAB)
                             for j in range(3)])
    qpos = np.where(tq < T, tq, 0)
    qlab = np.where(tq < T, -1, (tq - T) // LBL)
    qoff = np.where(tq < T, 0, (tq - T) % LBL)
    mq1 = allow(qpos, qlab, qoff).reshape(TT, 128, NQ)
    return maskd, mq1, tq


def _dr_pack(w, scale):
    """[512, N] f32 -> [128, 2*2*N] fp8 flat (a, s, N) DoubleRow layout."""
    N = w.shape[1]
    a = (w * scale).astype(FP8).reshape(2, 2, 128, N)
    return np.ascontiguousarray(
        a.transpose(2, 0, 1, 3).reshape(128, 4 * N))


def _dr_pack_w2(w, scale):
    """[1408, 512] f32 -> [128, 6*2*512] fp8 flat, HID padded to 1536."""
    wp = np.zeros((HA * 256, D), np.float32)
    wp[:HID] = w
    a = (wp * scale).astype(FP8).reshape(HA, 2, 128, D)
    return np.ascontiguousarray(
        a.transpose(2, 0, 1, 3).reshape(128, HA * 2 * D))


def _host_prep(inputs):
    """Build per-core in_maps from full inputs."""
    input_ids = np.asarray(inputs["input_ids"])
    label_ids = np.asarray(inputs["label_ids"])
    emb = np.asarray(inputs["emb"], dtype=np.float32)
    anw = np.asarray(inputs["attn_norm_w"], dtype=np.float32)
    fnw = np.asarray(inputs["ffn_norm_w"], dtype=np.float32)
    finw = np.asarray(inputs["final_norm_w"], dtype=np.float32)
    lm_head = np.asarray(inputs["lm_head"], dtype=np.float32)

    # fold norm weights into the consuming matmuls
    wq = np.asarray(inputs["wq"], np.float32) * anw[:, :, None]
    wk = np.asarray(inputs["wk"], np.float32) * anw[:, :, None]
    wv = np.asarray(inputs["wv"], np.float32) * anw[:, :, None]
    wo = np.asarray(inputs["wo"], np.float32)
    w1 = np.asarray(inputs["w1"], np.float32) * fnw[:, :, None]
    w3 = np.asarray(inputs["w3"], np.float32) * fnw[:, :, None]
    w2 = np.asarray(inputs["w2"], np.float32)
    lmh_f = lm_head * finw[:, None]

    suf_ids = label_ids.reshape(-1)

    # RoPE swap permutation on the output dim: rot = W[:, sigma]
    d_i = np.arange(D)
    sigma = (d_i // HD) * HD + ((d_i % HD) + HALF) % HD
    # per-layer fp8 weight blobs
    wb = np.zeros((NL, 128, WB_SZ), dtype=FP8)
    for l in range(NL):
        wb[l, :, WB_K:WB_K + 2048] = _dr_pack(wk[l], W_SC)
        wb[l, :, WB_KR:WB_KR + 2048] = _dr_pack(wk[l][:, sigma], W_SC)
        wb[l, :, WB_Q:WB_Q + 2048] = _dr_pack(wq[l], W_SC)
        wb[l, :, WB_QR:WB_QR + 2048] = _dr_pack(wq[l][:, sigma], W_SC)
        wb[l, :, WB_V:WB_V + 2048] = _dr_pack(wv[l], W_SC)
        wb[l, :, WB_O:WB_O + 2048] = _dr_pack(wo[l], W_SC)
        wb[l, :, WB_W1:WB_W1 + 4 * HID] = _dr_pack(w1[l], W_SC)
        wb[l, :, WB_W3:WB_W3 + 4 * HID] = _dr_pack(w3[l], W_SC)
        wb[l, :, WB_W2:WB_W2 + HA * 2 * D] = _dr_pack_w2(w2[l], W2_SC)

    # RoPE tables (1/W_SC folded in; q and k each carry one factor... both
    # raw projections are x W_SC, tables carry exactly 1/W_SC)
    pos = np.concatenate(
        [np.arange(T), np.tile(T + np.arange(LBL), NLAB)]).astype(np.float32)
    freqs = 1.0 / (10000.0 ** (np.arange(HALF, dtype=np.float32) / HALF))

    def rope_tabs(positions):
        ang = positions[None, :] * freqs[:, None]
        c = np.tile(np.cos(ang), (4, 1)) / W_SC
        s32 = np.sin(ang) / W_SC
        s = np.concatenate([-s32, s32, -s32, s32], 0)
        return c, s

    ctab, stab = rope_tabs(pos)
    maskd, mq1, tq = _build_masks()
    cqt, sqt = rope_tabs(pos[tq])

    # packed trimmed mask segments (see MSEG)
    maskc = np.concatenate([
        maskd[0][:, 0:128], maskd[1][:, 128:256], maskd[2][:, QC:384],
        maskd[3][:, 384:NTOK], maskd[4][:, 512:NTOK]], axis=1)  # [128, 536]
    # thin mask duplicated for head pairs: [128, 5, 32]
    maskqc = np.ascontiguousarray(
        np.tile(mq1, (1, 1, 2)).transpose(1, 0, 2))

    sel_cols = suf_ids.astype(np.int64)
    lmsel = np.ascontiguousarray(lmh_f[:, sel_cols])       # [512, 20]
    lmsel_p = lmsel.reshape(KT, 128, NSEL).transpose(1, 0, 2)

    cbs = np.zeros((128, CB_SZ), dtype=BF16)
    cbs[:, CB_C:CB_C + NTOK] = ctab.astype(BF16)
    cbs[:, CB_S:CB_S + NTOK] = stab.astype(BF16)
    cbs[:, CB_CQ:CB_CQ + NQ] = cqt.astype(BF16)
    cbs[:, CB_SQ:CB_SQ + NQ] = sqt.astype(BF16)
    cbs[:, CB_MD:CB_MD + MSEG_SZ] = maskc.astype(BF16)
    cbs[:, CB_MQ:CB_MQ + TT * 2 * NQ] = maskqc.reshape(128, -1).astype(BF16)
    cbs[:, CB_SEL:CB_SEL + KT * NSEL] = lmsel_p.reshape(128, -1).astype(BF16)

    in_maps = []
    for c in range(NCORES):
        b = c % B
        tok = np.concatenate([input_ids[b], suf_ids])
        x0 = np.ascontiguousarray(emb[tok].T) * RS
        x0p = x0.reshape(KT, 128, NTOK).transpose(1, 0, 2).astype(BF16)
        sh = np.zeros((D, NVCH * VCH), np.float32)
        sh[:, :VSH] = lmh_f[:, c * VSH:(c + 1) * VSH] * LMH_SCALE
        lmh8 = sh.astype(FP8).reshape(2, 2, 128, NVCH, VCH)
        lmh8 = np.ascontiguousarray(lmh8.transpose(2, 3, 0, 1, 4))
        m = dict(wb=wb, cb=cbs, x0T=np.ascontiguousarray(x0p), lmh=lmh8)
        in_maps.append(m)
    return in_maps


def _host_combine(results):
    """Combine per-core partial sumexp + selected logits into [B, NLAB]."""
    se = np.zeros((NROW,), dtype=np.float64)
    for c in range(NCORES):
        # each padded column contributes exp(0)=1 to every row's partial
        se += np.asarray(results[c]["se_out"], np.float64).sum(axis=1) - VPAD
    lse = np.log(se)
    sel = np.asarray(results[0]["sel_out"], np.float64)    # [80, 20]
    rows = np.arange(NROW)
    bb = rows // (NLAB * LBL)
    ll = (rows % (NLAB * LBL)) // LBL
    jj = rows % LBL
    lp = sel[rows, ll * LBL + jj] - lse
    out = np.zeros((B, NLAB), dtype=np.float64)
    np.add.at(out, (bb, ll), lp)
    return out.astype(np.float32)


def kernel(**inputs):
    nc = _get_nc()
    in_maps = _host_prep(inputs)
    res = run_bass_kernel_spmd(
        nc, in_maps, core_ids=list(range(NCORES)),
        trace=_CACHE.get("trace", False),
    )
    _CACHE["last_results"] = res
    return _host_combine(res.results)


# revision 44
# speedup vs baseline: 1.0545x; 1.0545x over previous
"""Trainium2 Bass kernel for nn_LlamaEmbeddingClassifier.

Model: 2-layer Llama (D=512, 8 heads x 64, HID=1408, RoPE, RMSNorm) scoring
B=4 prompts against NLAB=5 label continuations (LBL=4 tokens) with an
lm_head over V=128000.

Strategy (8 NeuronCores, single SPMD launch):
  - Packed 528-token sequence [508 prefix | 5 x 4-token suffixes] with a
    custom attention mask; core c handles batch row (c % 4); cores 4-7
    duplicate 0-3 (SPMD program is uniform).  Layer 2 runs full K/V but a
    thin 16-query path for attention/MLP.
  - All weight matmuls (wq/wk/wv/wo/w1/w3/w2) run in fp8-e4m3 DoubleRow
    perf mode (2x PE throughput, half the weight HBM traffic).  The
    residual stream is stored scaled x64 so host-prescaled fp8 weights
    (x64) need no on-device compensation: RMSNorm is scale-invariant
    (eps folded as 64^2*eps), and both wo/w2 outputs land back on the
    x64 scale.  RoPE is applied via dual projections (W and W*P^T both
    in the fp8 weight blob) - no on-device transpose/permute matmul.
  - Attention probabilities are stored fp8; PV runs DoubleRow over
    key-tile pairs.  Softmax denominators come free via a ones-column in
    the value tile; exp is un-normalized (scores bounded ~|1.5|).
  - Per-layer weights are packed into one fp8 DRAM blob -> one big DMA
    per layer (512B+ contiguous runs, minimal HWDGE occupancy).  The
    fp8 lm_head shard (vocab/8 per core, padded to 16384 cols so chunk
    runs are exactly 512B) is fully resident in SBUF, prefetched during
    the transformer; the host subtracts the known exp(0) pad
    contribution from the sumexp.
  - The 16 final hidden rows per batch are AllGathered (tiny) so every
    core scores all 80 rows over its vocab shard.  Label-token logits
    come from a bf16 side matmul (lmsel) to keep the accuracy-critical
    path out of fp8.  Host combines: logsumexp across shards,
    lp = sel_logit - lse, summed per (batch,label).
"""

import math
import os
import sys
from contextlib import ExitStack

for _p in ("/opt/trn_rl_repo", "/root/.axon_site/_ro/trn_rl_repo"):
    if os.path.isdir(_p) and _p not in sys.path:
        sys.path.insert(0, _p)

import ml_dtypes
import numpy as np

import concourse.bass as bass
import concourse.tile as tile
from concourse import bacc, mybir
from concourse.bass_utils import run_bass_kernel_spmd

BF16 = ml_dtypes.bfloat16
FP8 = np.dtype(ml_dtypes.float8_e4m3)

# Problem dims (hardcoded per contract)
V, D, NH, NL, HID = 128000, 512, 8, 2, 1408
HD, HALF = 64, 32
B, T, NLAB, LBL = 4, 508, 5, 4
EPS = 1e-5
NCORES = 8
SUF = NLAB * LBL            # 20 suffix tokens
NTOK = T + SUF              # 528 packed tokens
KT = D // 128               # 4 K-tiles over D
TT = (NTOK + 127) // 128    # 5 token tiles (last has 16 rows)
TP = 3                      # key-tile pairs for DoubleRow PV
HT = HID // 128             # 11 tiles over HID
HA = 6                      # DoubleRow passes over padded HID (1536)
NROW = B * NLAB * LBL       # 80 scoring rows
NSEL = NLAB * LBL           # 20 selected lm_head columns
NQ = 1 + NLAB * (LBL - 1)   # 16 thin-path query positions
VSH = V // NCORES           # 16000 vocab shard per core
VCH = 512                   # vocab chunk (512B fp8 runs, 2KB psum bank)
NVCH = 32                   # chunks over the padded shard
VPAD = NVCH * VCH - VSH     # 384 zero-padded columns -> exp(0)=1 each
QC = 264                    # q chunk (2 chunks of 264 = 528)

RS = 64.0                   # residual-stream scale (h stores 64*h_true)
W_SC = 64.0                 # host scale for wq/wk/wv/wo/w1/w3 (fp8 range)
W2_SC = 16.0                # host scale for w2
G1_SC = 4.0                 # on-device scale of stored g1 (W2_SC*G1_SC=RS)
EPS_S = EPS * RS * RS       # rms eps on the scaled residual
LMH_SCALE = 32.0            # host premultiply of fp8 lm_head
HS_SCALE = 4.0              # device premultiply of fp8 hs copies

# causal block structure: q-chunk 0 (q<264) only sees k-tiles 0,1; q-chunk 1
# sees all.  (Queries 256..263 lose keys 256..263 - tiny, within tolerance.)
CH_MTS = {0: (0, 1), 1: (0, 1, 2, 3, 4)}
CH_TPS = {0: (0,), 1: (0, 1, 2)}   # same structure as key-tile pairs
# exp only over columns that are not fully causally masked
EXPR = {(0, 0): (0, QC), (1, 0): (128, QC),
        (0, 1): (QC, NTOK), (1, 1): (QC, NTOK), (2, 1): (QC, NTOK),
        (3, 1): (384, NTOK), (4, 1): (512, NTOK)}
# mask-multiply ranges (within the exp'd region) + offset into packed maskc
MSEG = {0: (0, 128, 0), 1: (128, 256, 128), 2: (QC, 384, 256),
        3: (384, NTOK, 376), 4: (512, NTOK, 520)}
MSEG_SZ = 536

# fp8 weight-blob element offsets (per partition), layer-major
WB_K, WB_KR, WB_Q, WB_QR = 0, 2048, 4096, 6144
WB_V, WB_O = 8192, 10240
WB_W1, WB_W3, WB_W2 = 12288, 17920, 23552
WB_SZ = 29696
# consts blob (bf16) element offsets
CB_C, CB_S = 0, NTOK
CB_CQ, CB_SQ = 2 * NTOK, 2 * NTOK + NQ
CB_MD = 2 * NTOK + 2 * NQ                 # packed trimmed mask segments
CB_MQ = CB_MD + MSEG_SZ                   # [5, 32] head-pair thin mask
CB_SEL = CB_MQ + TT * 2 * NQ              # [4, 20] lmsel
CB_SZ = CB_SEL + KT * NSEL

_CACHE = {}


def _tok_rows(tt):
    return min(128, NTOK - tt * 128)


def build_nc(use_collective=True):
    nc = bacc.Bacc("TRN2", num_devices=NCORES)
    f32, bf16, fp8 = mybir.dt.float32, mybir.dt.bfloat16, mybir.dt.float8e4
    DR = mybir.MatmulPerfMode.DoubleRow

    # ---- I/O ----
    x0T = nc.dram_tensor("x0T", [128, KT, NTOK], bf16,
                         kind="ExternalInput")
    wb = nc.dram_tensor("wb", [NL, 128, WB_SZ], fp8, kind="ExternalInput")
    cb = nc.dram_tensor("cb", [128, CB_SZ], bf16, kind="ExternalInput")
    lmh = nc.dram_tensor("lmh", [128, NVCH, 2, 2, VCH], fp8,
                         kind="ExternalInput")

    se_out = nc.dram_tensor("se_out", [NROW, NVCH], bf16,
                            kind="ExternalOutput")
    sel_out = nc.dram_tensor("sel_out", [NROW, NSEL], f32,
                             kind="ExternalOutput")

    with tile.TileContext(nc) as tc, ExitStack() as ctx:
        consts = ctx.enter_context(tc.tile_pool(name="consts", bufs=1))
        wpool = ctx.enter_context(tc.tile_pool(name="weights", bufs=1))
        lpool = ctx.enter_context(tc.tile_pool(name="lmh", bufs=1))
        acts = ctx.enter_context(tc.tile_pool(name="acts", bufs=1))
        scr = ctx.enter_context(tc.tile_pool(name="scratch", bufs=6))
        ppool = ctx.enter_context(tc.tile_pool(name="p", bufs=2))
        psum = ctx.enter_context(tc.tile_pool(name="psum", bufs=7,
                                              space="PSUM"))
        psmall = ctx.enter_context(tc.tile_pool(name="psmall", bufs=1,
                                                space="PSUM"))
        dram = ctx.enter_context(tc.tile_pool(name="dram", bufs=1,
                                              space="DRAM"))

        # ---- resident tiles ----
        h = acts.tile([128, KT, NTOK], bf16, name="h")
        cbs = consts.tile([128, CB_SZ], bf16, name="cbs")
        wsb = wpool.tile([128, NL, WB_SZ], fp8, name="wsb")
        lsb = lpool.tile([128, NVCH, 2, 2, VCH], fp8, name="lsb")

        # ---- input DMAs, in pipeline order (single shared DMA pipe) ----
        nc.sync.dma_start(out=h[:], in_=x0T[:])
        nc.sync.dma_start(out=cbs[:], in_=cb[:])
        nc.sync.dma_start(out=wsb[:, 0, :WB_V], in_=wb[0][:, :WB_V])
        nc.sync.dma_start(out=wsb[:, 0, WB_V:], in_=wb[0][:, WB_V:])
        nc.sync.dma_start(out=wsb[:, 1, :], in_=wb[1][:])
        nc.sync.dma_start(out=lsb[:, :NVCH // 2], in_=lmh[:, :NVCH // 2])
        nc.sync.dma_start(out=lsb[:, NVCH // 2:], in_=lmh[:, NVCH // 2:])

        # ---- const views ----
        C128 = cbs[:, CB_C:CB_C + NTOK]
        S128 = cbs[:, CB_S:CB_S + NTOK]
        Cq = cbs[:, CB_CQ:CB_CQ + NQ]
        Sq = cbs[:, CB_SQ:CB_SQ + NQ]
        maskc = cbs[:, CB_MD:CB_MD + MSEG_SZ]
        maskq = cbs[:, CB_MQ:CB_MQ + TT * 2 * NQ].rearrange(
            "p (t q) -> p t q", t=TT)
        lmsel_sb = cbs[:, CB_SEL:CB_SEL + KT * NSEL].rearrange(
            "p (k c) -> p k c", k=KT)

        def wv_(l, off, n, a=2):
            return wsb[:, l, off:off + a * 2 * n].rearrange(
                "p (a s n) -> p a s n", a=a, s=2)

        ones_col = consts.tile([128, 1], bf16)
        nc.vector.memset(ones_col, 1.0)
        eps_sb = consts.tile([1, 1], f32)
        nc.vector.memset(eps_sb, float(EPS_S))

        # activations
        xn8 = acts.tile([128, 2, 2, NTOK], fp8, name="xn8")
        xn8b = acts.tile([128, 2, 2, NTOK], fp8, name="xn8b")
        kTt = acts.tile([128, KT, NTOK], bf16, name="kTt")
        qT = acts.tile([128, KT, NTOK], bf16, name="qT")
        oT8 = acts.tile([128, 2, 2, NTOK], fp8, name="oT8")
        g1 = acts.tile([128, HA, 2, NTOK], fp8, name="g1")
        # v with interleaved ones column plus one pad column (so the
        # DoubleRow lhsT pair-stride is 528 = a multiple of 16 bytes), in
        # key-tile-pair layout: [keys(128), pair, slot, 8 x (64 v | one |pad)]
        HDP = HD + 2
        v_aug = acts.tile([128, TP, 2, NH * HDP], fp8, name="v_aug")
        v5 = v_aug.rearrange("p t s (h c) -> p t s h c", c=HDP)
        nc.gpsimd.memset(v5[:, :, :, :, HD:], 1.0)    # ones (+pad) columns
        nc.gpsimd.memset(v5[:, 2, 1, :, :HD], 0.0)    # pad slot (tile 5)
        nc.gpsimd.memset(v5[:, 2, 0, :, :HD], 0.0)    # tile-4 (rows 0:16 get
        # overwritten by the real v projection; tail rows stay zero)
        nc.gpsimd.memset(g1[:, HA - 1, 1, :], 0.0)    # padded HID slot

        hq = acts.tile([128, KT, NQ], bf16, name="hq")
        xnq8 = acts.tile([128, 2, 2, NQ], fp8, name="xnq8")
        xnq8b = acts.tile([128, 2, 2, NQ], fp8, name="xnq8b")
        qTq = acts.tile([128, KT, NQ], bf16, name="qTq")
        oTq8 = acts.tile([128, 2, 2, NQ], fp8, name="oTq8")
        g1q = acts.tile([128, HA, 2, NQ], fp8, name="g1q")
        nc.gpsimd.memset(g1q[:, HA - 1, 1, :], 0.0)

        FULL_CH = ((0, QC), (QC, NTOK))
        THIN_CH = ((0, NQ),)

        def rms(src, dest, n, chunks, sq_eng, mul_eng=None):
            mul_eng = mul_eng or nc.vector
            """dest (fp8, DR layout) = src * rsqrt(mean_D(src^2)+eps').

            Emitted per column-chunk so downstream projections of chunk 0
            can start while chunk 1 is still normalizing.
            """
            for c0, c1 in chunks:
                w = c1 - c0
                sq = scr.tile([128, KT, QC], bf16, name="sq", bufs=2)
                for kt in range(KT):
                    sq_eng.tensor_mul(out=sq[:, kt, :w],
                                      in0=src[:, kt, c0:c1],
                                      in1=src[:, kt, c0:c1])
                ss = psmall.tile([1, QC], f32, name="ss", tag="small")
                for kt in range(KT):
                    nc.tensor.matmul(ss[:, :w], ones_col[:], sq[:, kt, :w],
                                     start=(kt == 0), stop=(kt == KT - 1))
                # rsqrt(x) = exp(-0.5*ln(x)); Ln/Exp share an act table
                lnb = scr.tile([1, QC], f32, name="lnb")
                nc.scalar.activation(out=lnb[:, :w], in_=ss[:, :w],
                                     func=mybir.ActivationFunctionType.Ln,
                                     scale=1.0 / D, bias=eps_sb[:])
                rstd = scr.tile([1, QC], bf16, name="rstd")
                nc.scalar.activation(out=rstd[:, :w], in_=lnb[:, :w],
                                     func=mybir.ActivationFunctionType.Exp,
                                     scale=-0.5)
                rb = scr.tile([128, QC], bf16, name="rms_rb", bufs=2)
                nc.gpsimd.partition_broadcast(rb[:, :w], rstd[:, :w])
                for kt in range(KT):
                    mul_eng.tensor_mul(out=dest[:, kt // 2, kt % 2, c0:c1],
                                       in0=src[:, kt, c0:c1],
                                       in1=rb[:, :w])

        def proj_rope(specs, xn, chunks, ctb, stb, mts=None):
            """dest[Dout, n] (bf16) = rope((xn @ W).T) via two fp8 DR projs.

            specs: list of (dest, w_raw_view, w_rot_view); tiles of the
            different projections are interleaved for engine overlap.
            """
            for c0, c1 in chunks:
                n = c1 - c0
                for mt in (range(KT) if mts is None else mts):
                    msl = slice(mt * 128, (mt + 1) * 128)
                    for dest, wv_raw, wv_rot in specs:
                        psA = psum.tile([128, QC], f32, name="pjA", tag="mm")
                        psB = psum.tile([128, QC], f32, name="pjB", tag="mm")
                        for a in range(2):
                            nc.tensor.matmul(psA[:, :n], wv_raw[:, a, :, msl],
                                             xn[:, a, :, c0:c1],
                                             start=(a == 0), stop=(a == 1),
                                             perf_mode=DR)
                        for a in range(2):
                            nc.tensor.matmul(psB[:, :n], wv_rot[:, a, :, msl],
                                             xn[:, a, :, c0:c1],
                                             start=(a == 0), stop=(a == 1),
                                             perf_mode=DR)
                        t1 = scr.tile([128, QC], bf16, name="rope_t1")
                        nc.vector.tensor_mul(out=t1[:, :n], in0=psA[:, :n],
                                             in1=ctb[:, c0:c1])
                        t2 = scr.tile([128, QC], bf16, name="rope_t2")
                        nc.vector.tensor_mul(out=t2[:, :n], in0=psB[:, :n],
                                             in1=stb[:, c0:c1])
                        nc.gpsimd.tensor_add(out=dest[:, mt, c0:c1],
                                             in0=t1[:, :n], in1=t2[:, :n])

        def v_proj(wv_v, xn, n_tiles, ncols):
            """v_aug[:, tp, s, h, :HD] = (xn @ Wv).T / RS in pair layout."""
            for mt in range(n_tiles):
                mr = min(128, ncols - mt * 128)
                ps = psum.tile([128, D], f32, name="v_ps", tag="mm")
                if mr == 128:
                    for a in range(2):
                        nc.tensor.matmul(
                            ps[:mr, :], xn[:, a, :, mt * 128:mt * 128 + mr],
                            wv_v[:, a],
                            start=(a == 0), stop=(a == 1), perf_mode=DR)
                else:  # tail: plain fp8 (DR needs a full 128-col stationary)
                    for i, (a, s_) in enumerate(
                            [(a, s_) for a in range(2) for s_ in range(2)]):
                        nc.tensor.matmul(
                            ps[:mr, :], xn[:, a, s_, mt * 128:mt * 128 + mr],
                            wv_v[:, a, s_],
                            start=(i == 0), stop=(i == 3))
                nc.vector.tensor_scalar_mul(
                    out=v5[:mr, mt // 2, mt % 2, :, :HD],
                    in0=ps.rearrange("p (h c) -> p h c", c=HD)[:mr],
                    scalar1=1.0 / RS)

        def attn_norm(po, a, s, b, dest8, cs, n):
            """dest8 = po[:HD]/po[HD] (softmax denominator), fp8 out."""
            rs_t = scr.tile([1, QC], f32, name="attn_rs")
            nc.vector.reciprocal(out=rs_t[:, :n], in_=po[HD:HD + 1, :n])
            rb_t = scr.tile([64, QC], f32, name="attn_rb")
            nc.gpsimd.partition_broadcast(rb_t[:, :n], rs_t[:, :n])
            nc.vector.tensor_mul(
                out=dest8[64 * b:64 * b + 64, a, s, cs],
                in0=po[:HD, :n], in1=rb_t[:, :n])

        def attn_head_full(hh):
                tq = hh // 2
                rq = slice(64 * (hh % 2), 64 * (hh % 2) + 64)
                a, s, bb = hh // 4, (hh // 2) % 2, hh % 2
                p_sb = ppool.tile([128, TP, 2, NTOK], fp8, name="p_sb")
                if hh < 2:  # zero exp-trimmed regions in both buffers
                    nc.gpsimd.memset(p_sb[:, 0, 1, 0:128], 0.0)
                    nc.gpsimd.memset(p_sb[:, 1, 1, QC:384], 0.0)
                    nc.gpsimd.memset(p_sb[:, 2, 0, QC:], 0.0)
                    nc.gpsimd.memset(p_sb[:, 2, 1, QC:], 0.0)
                for mt in range(TT):
                    mr = _tok_rows(mt)
                    for ch in range(2):
                        if mt not in CH_MTS[ch]:
                            continue
                        e0, e1 = EXPR[(mt, ch)]
                        cs = slice(ch * QC, (ch + 1) * QC)
                        ps = psum.tile([128, QC], f32, name="score_ps",
                                       tag="mm")
                        nc.tensor.matmul(
                            ps[:mr, :e1 - e0],
                            kTt[rq, tq, mt * 128:mt * 128 + mr],
                            qT[rq, tq, e0:e1], start=True, stop=True)
                        nc.scalar.activation(
                            out=p_sb[:mr, mt // 2, mt % 2, e0:e1],
                            in_=ps[:mr, :e1 - e0],
                            func=mybir.ActivationFunctionType.Exp,
                            scale=1.0 / math.sqrt(HD))
                    m0, m1, mo = MSEG[mt]
                    nc.gpsimd.tensor_mul(
                        out=p_sb[:mr, mt // 2, mt % 2, m0:m1],
                        in0=p_sb[:mr, mt // 2, mt % 2, m0:m1],
                        in1=maskc[:mr, mo:mo + m1 - m0])
                for ch in range(2):
                    cs = slice(ch * QC, (ch + 1) * QC)
                    tps = CH_TPS[ch]
                    po = psum.tile([128, QC], f32, name="pv_ps", tag="mm")
                    for i, tp in enumerate(tps):
                        nc.tensor.matmul(
                            po[:HD + 2, :],
                            v_aug[:, tp, :, hh * HDP:(hh + 1) * HDP],
                            p_sb[:, tp, :, cs],
                            start=(i == 0), stop=(i == len(tps) - 1),
                            perf_mode=DR)
                    attn_norm(po, a, s, bb, oT8, cs, QC)

        def build_qz():
            # head-pair batching: kTt's 128 partitions hold two heads; the
            # query block is zero-padded so one matmul yields both heads'
            # scores side by side ([mr, 32], offset 0 - HW-safe).
            qz = scr.tile([128, KT, 2 * NQ], bf16, name="qz", bufs=1)
            nc.vector.memset(qz[:], 0.0)
            for tq in range(KT):
                nc.vector.tensor_copy(out=qz[0:64, tq, 0:NQ],
                                      in_=qTq[0:64, tq, :])
                nc.vector.tensor_copy(out=qz[64:128, tq, NQ:2 * NQ],
                                      in_=qTq[64:128, tq, :])
            return qz

        def attn_group_thin(tq, qz):
                p_sb = ppool.tile([128, TP, 2, 2 * NQ], fp8, name="pq_sb")
                if tq < 2:
                    nc.gpsimd.memset(p_sb[:, 2, 1, :], 0.0)
                    nc.gpsimd.memset(p_sb[:, 2, 0, :], 0.0)
                for mt in range(TT):
                    mr = _tok_rows(mt)
                    ps = psum.tile([128, QC], f32, name="score_ps", tag="mm")
                    nc.tensor.matmul(
                        ps[:mr, :2 * NQ],
                        kTt[:, tq, mt * 128:mt * 128 + mr],
                        qz[:, tq, :], start=True, stop=True)
                    nc.scalar.activation(
                        out=p_sb[:mr, mt // 2, mt % 2, :],
                        in_=ps[:mr, :2 * NQ],
                        func=mybir.ActivationFunctionType.Exp,
                        scale=1.0 / math.sqrt(HD))
                    nc.gpsimd.tensor_mul(
                        out=p_sb[:mr, mt // 2, mt % 2, :],
                        in0=p_sb[:mr, mt // 2, mt % 2, :],
                        in1=maskq[:mr, mt, :])
                for half in range(2):
                    hh = 2 * tq + half
                    a, s, bb = hh // 4, (hh // 2) % 2, hh % 2
                    nsl = slice(half * NQ, (half + 1) * NQ)
                    po = psum.tile([128, QC], f32, name="pv_ps", tag="mm")
                    for tp in range(TP):
                        nc.tensor.matmul(
                            po[:HD + 2, :NQ],
                            v_aug[:, tp, :, hh * HDP:(hh + 1) * HDP],
                            p_sb[:, tp, :, nsl],
                            start=(tp == 0), stop=(tp == TP - 1),
                            perf_mode=DR)
                    attn_norm(po, a, s, bb, oTq8, slice(0, NQ), NQ)

        def accum_proj_dr(w_view, npass, src8, dest, chunks, add_eng):
            """dest += (src8 DR-matmul w).T ; w_view [128, npass, 2, Dout]."""
            for c0, c1 in chunks:
                n = c1 - c0
                for mt in range(KT):
                    msl = slice(mt * 128, (mt + 1) * 128)
                    ps = psum.tile([128, QC], f32, name="acc_ps", tag="mm")
                    for a in range(npass):
                        nc.tensor.matmul(
                            ps[:, :n], w_view[:, a, :, msl],
                            src8[:, a, :, c0:c1],
                            start=(a == 0), stop=(a == npass - 1),
                            perf_mode=DR)
                    add_eng.tensor_add(out=dest[:, mt, c0:c1],
                                       in0=dest[:, mt, c0:c1], in1=ps[:, :n])

        def mlp(l, xn, gdest, chunks, dest, n_ht, add_eng):
            w1v, w3v = wv_(l, WB_W1, HID), wv_(l, WB_W3, HID)
            w2v = wv_(l, WB_W2, D, a=HA)
            for ci, (c0, c1) in enumerate(chunks):
                n = c1 - c0
                for mt in range(n_ht):
                    msl = slice(mt * 128, (mt + 1) * 128)
                    ps3 = psum.tile([128, QC], f32, name="g3_ps", tag="mm")
                    for a in range(2):
                        nc.tensor.matmul(ps3[:, :n], w3v[:, a, :, msl],
                                         xn[:, a, :, c0:c1],
                                         start=(a == 0), stop=(a == 1),
                                         perf_mode=DR)
                    ps1 = psum.tile([128, QC], f32, name="g1_ps", tag="mm")
                    for a in range(2):
                        nc.tensor.matmul(ps1[:, :n], w1v[:, a, :, msl],
                                         xn[:, a, :, c0:c1],
                                         start=(a == 0), stop=(a == 1),
                                         perf_mode=DR)
                    tsil = scr.tile([128, QC], bf16, name="tsil")
                    nc.scalar.activation(
                        out=tsil[:, :n], in_=ps1[:, :n],
                        func=mybir.ActivationFunctionType.Silu,
                        scale=1.0 / W_SC)
                    # g1 = (ps3 * G1_SC/W_SC) * silu  (fused; DVE - reads PSUM)
                    nc.vector.scalar_tensor_tensor(
                        out=gdest[:, mt // 2, mt % 2, c0:c1],
                        in0=ps3[:, :n], scalar=G1_SC / W_SC,
                        in1=tsil[:, :n],
                        op0=mybir.AluOpType.mult, op1=mybir.AluOpType.mult)
            accum_proj_dr(w2v, HA, gdest, dest, chunks, add_eng)

        def gather_q(dest, src):
            """cols: 0 <- 507; 1+3l+j <- 508+4l+j (j=0..2); [128, k, cols]"""
            for kt in range(src.shape[1]):
                nc.vector.tensor_copy(out=dest[:, kt, 0:1],
                                      in_=src[:, kt, T - 1:T])
                nc.vector.tensor_copy(
                    out=dest[:, kt, 1:NQ].rearrange("p (l s) -> p l s", s=3),
                    in_=src[:, kt, T:T + SUF].rearrange(
                        "p (l s) -> p l s", s=LBL)[:, :, 0:3])

        def gather_q4(dest, src):
            for a in range(2):
                for s_ in range(2):
                    nc.vector.tensor_copy(out=dest[:, a, s_, 0:1],
                                          in_=src[:, a, s_, T - 1:T])
                    nc.vector.tensor_copy(
                        out=dest[:, a, s_, 1:NQ].rearrange(
                            "p (l s) -> p l s", s=3),
                        in_=src[:, a, s_, T:T + SUF].rearrange(
                            "p (l s) -> p l s", s=LBL)[:, :, 0:3])

        # ================= transformer =================
        # Attention of heads (2t, 2t+1) only needs k/q tile t, so emit each
        # projection tile followed immediately by its two heads - the PE's
        # in-order queue + psum backpressure otherwise delays the first
        # score matmul until every projection tile has drained.
        for l in range(NL):
            full = l < NL - 1
            rms(h, xn8, NTOK, FULL_CH, nc.vector if full else nc.gpsimd,
                nc.vector if full else nc.gpsimd)
            if full:
                specs = [(kTt, wv_(l, WB_K, D), wv_(l, WB_KR, D)),
                         (qT, wv_(l, WB_Q, D), wv_(l, WB_QR, D))]
                proj_rope(specs, xn8, FULL_CH, C128, S128)
                v_proj(wv_(l, WB_V, D), xn8, TT, NTOK)
                for hh in range(NH):
                    attn_head_full(hh)
                accum_proj_dr(wv_(l, WB_O, D), 2, oT8, h, FULL_CH, nc.vector)
                rms(h, xn8b, NTOK, FULL_CH, nc.gpsimd, nc.gpsimd)
                mlp(l, xn8b, g1, FULL_CH, h, HT, nc.vector)
            else:
                gather_q(hq, h)
                gather_q4(xnq8, xn8)
                proj_rope([(qTq, wv_(l, WB_Q, D), wv_(l, WB_QR, D))],
                          xnq8, THIN_CH, Cq, Sq)
                qz = build_qz()
                kspec = [(kTt, wv_(l, WB_K, D), wv_(l, WB_KR, D))]
                proj_rope(kspec, xn8, FULL_CH, C128, S128)
                v_proj(wv_(l, WB_V, D), xn8, TT, NTOK)
                for tq in range(KT):
                    attn_group_thin(tq, qz)
                accum_proj_dr(wv_(l, WB_O, D), 2, oTq8, hq, THIN_CH,
                              nc.vector)
                rms(hq, xnq8b, NQ, THIN_CH, nc.vector)
                mlp(l, xnq8b, g1q, THIN_CH, hq, HT, nc.vector)

        # ============ final norm + extract + AllGather ============
        xnf = scr.tile([128, KT, NQ], bf16, name="xnf")
        # bf16 out (not fp8): feeds the accuracy-critical lmsel path
        sqf = scr.tile([128, KT, NQ], bf16, name="sqf", bufs=1)
        for kt in range(KT):
            nc.vector.tensor_mul(out=sqf[:, kt, :], in0=hq[:, kt, :],
                                 in1=hq[:, kt, :])
        ssf = psmall.tile([1, QC], f32, name="ss", tag="small")
        for kt in range(KT):
            nc.tensor.matmul(ssf[:, :NQ], ones_col[:], sqf[:, kt, :],
                             start=(kt == 0), stop=(kt == KT - 1))
        lnf = scr.tile([1, QC], f32, name="lnb")
        nc.scalar.activation(out=lnf[:, :NQ], in_=ssf[:, :NQ],
                             func=mybir.ActivationFunctionType.Ln,
                             scale=1.0 / D, bias=eps_sb[:])
        rstdf = scr.tile([1, NQ], bf16, name="rstdf")
        nc.scalar.activation(out=rstdf[:], in_=lnf[:, :NQ],
                             func=mybir.ActivationFunctionType.Exp,
                             scale=-0.5)
        rbf = scr.tile([128, NQ], bf16, name="rbf")
        nc.gpsimd.partition_broadcast(rbf[:], rstdf[:])
        for kt in range(KT):
            nc.vector.tensor_mul(out=xnf[:, kt, :], in0=hq[:, kt, :],
                                 in1=rbf[:])

        hsT_own = acts.tile([128, KT, NSEL], bf16, name="hsT_own")
        for kt in range(KT):
            eng = nc.vector if kt % 2 == 0 else nc.gpsimd
            for ll in range(NLAB):
                eng.tensor_copy(
                    out=hsT_own[:, kt, ll * LBL:ll * LBL + 1],
                    in_=xnf[:, kt, 0:1])
            eng.tensor_copy(
                out=hsT_own.rearrange("p k (l s) -> p k l s", s=LBL)[
                    :, kt, :, 1:LBL],
                in_=xnf[:, kt, 1:NQ].rearrange("p (l s) -> p l s", s=3))

        cc_in = dram.tile([D, NSEL], bf16)
        cc_out = dram.tile([NCORES * D, NSEL], bf16)
        nc.sync.dma_start(
            out=cc_in.rearrange("(k p) c -> p k c", p=128), in_=hsT_own[:])
        if use_collective:
            nc.gpsimd.collective_compute(
                "AllGather",
                mybir.AluOpType.bypass,
                replica_groups=[list(range(NCORES))],
                ins=[cc_in.opt()],
                outs=[cc_out.opt()],
            )
        else:  # timeline-sim variant: emulate with local copies, spread
            # across four queues so the copies run concurrently
            qs = [nc.sync, nc.scalar, nc.gpsimd]
            for r in range(NCORES):
                qs[r % 3].dma_start(
                    out=cc_out[r * D:(r + 1) * D, :], in_=cc_in[:])

        hsT_all = acts.tile([128, KT, B, NSEL], bf16, name="hsT_all")
        cc_view = cc_out.rearrange("(b k p) c -> p k b c", b=NCORES, p=128)
        qs2 = [nc.sync, nc.scalar, nc.gpsimd, nc.scalar]
        for kt in range(KT):
            qs2[kt].dma_start(out=hsT_all[:, kt], in_=cc_view[:, kt, 0:B, :])

        # ================= lm_head phase =================
        hs8 = acts.tile([128, 2, 2, B * NSEL], fp8, name="hs8")
        for kt in range(KT):
            nc.vector.tensor_scalar_mul(
                out=hs8[:, kt // 2, kt % 2, :],
                in0=hsT_all.rearrange("p k b c -> p k (b c)")[:, kt, :],
                scalar1=HS_SCALE)
        se_sb = acts.tile([NROW, NVCH], bf16, name="se_sb")
        escale = 1.0 / (LMH_SCALE * HS_SCALE)
        for j in range(NVCH):
            pl = psum.tile([NROW, VCH], f32, name="lm_ps", tag="mm")
            for a in range(2):
                nc.tensor.matmul(pl[:], hs8[:, a], lsb[:, j, a],
                                 start=(a == 0), stop=(a == 1),
                                 perf_mode=DR)
            esc = scr.tile([NROW, VCH], bf16, name="esc", bufs=3)
            nc.scalar.activation(
                out=esc[:], in_=pl[:],
                func=mybir.ActivationFunctionType.Exp,
                scale=escale)
            # row-sum on the (otherwise idle) DVE instead of the scalar
            # engine's accumulator read
            with nc.allow_low_precision("bf16 partial sumexp: one rounding "
                                        "per 512-col chunk, ~0.07% on lse"):
                nc.vector.tensor_reduce(out=se_sb[:, j:j + 1], in_=esc[:],
                                        axis=mybir.AxisListType.X,
                                        op=mybir.AluOpType.add)
        nc.sync.dma_start(out=se_out[:], in_=se_sb[:])

        psel = psmall.tile([NROW, NSEL], f32, name="sel_ps", tag="small")
        for kt in range(KT):
            nc.tensor.matmul(
                psel[:],
                hsT_all.rearrange("p k b c -> p k (b c)")[:, kt, :],
                lmsel_sb[:, kt, :],
                start=(kt == 0), stop=(kt == KT - 1))
        sel_sb = scr.tile([NROW, NSEL], f32, name="sel_sb")
        nc.scalar.copy(out=sel_sb[:], in_=psel[:])
        nc.sync.dma_start(out=sel_out[:], in_=sel_sb[:])

    nc.finalize()
    return nc


def _get_nc():
    if "nc" not in _CACHE:
        _CACHE["nc"] = build_nc()
    return _CACHE["nc"]


def _build_masks():
    """full mask [TT,128,NTOK] and thin mask [TT,128,NQ] over (k, q)."""
    k_idx = np.arange(TT * 128)
    kpos = np.where(k_idx < T, k_idx, 0)
    klab = np.where(k_idx < T, -1, (k_idx - T) // LBL)
    koff = np.where(k_idx < T, 0, (k_idx - T) % LBL)
    kvalid = k_idx < NTOK

    def allow(qpos, qlab, qoff):
        kp = kpos[:, None]; kl = klab[:, None]; ko = koff[:, None]
        prefix_k = kl == -1
        prefix_q = (qlab == -1)[None, :]
        a = np.where(
            prefix_q,
            prefix_k & (kp <= qpos[None, :]),
            prefix_k | ((kl == qlab[None, :]) & (ko <= qoff[None, :])),
        )
        return (a & kvalid[:, None]).astype(np.float32)

    q_idx = np.arange(NTOK)
    qpos = np.where(q_idx < T, q_idx, 0)
    qlab = np.where(q_idx < T, -1, (q_idx - T) // LBL)
    qoff = np.where(q_idx < T, 0, (q_idx - T) % LBL)
    maskd = allow(qpos, qlab, qoff).reshape(TT, 128, NTOK)

    tq = np.array([T - 1] + [T + 4 * l + j for l in range(NLAB)
                             for j in range(3)])
    qpos = np.where(tq < T, tq, 0)
    qlab = np.where(tq < T, -1, (tq - T) // LBL)
    qoff = np.where(tq < T, 0, (tq - T) % LBL)
    mq1 = allow(qpos, qlab, qoff).reshape(TT, 128, NQ)
    return maskd, mq1, tq


def _dr_pack(w, scale):
    """[512, N] f32 -> [128, 2*2*N] fp8 flat (a, s, N) DoubleRow layout."""
    N = w.shape[1]
    a = (w * scale).astype(FP8).reshape(2, 2, 128, N)
    return np.ascontiguousarray(
        a.transpose(2, 0, 1, 3).reshape(128, 4 * N))


def _dr_pack_w2(w, scale):
    """[1408, 512] f32 -> [128, 6*2*512] fp8 flat, HID padded to 1536."""
    wp = np.zeros((HA * 256, D), np.float32)
    wp[:HID] = w
    a = (wp * scale).astype(FP8).reshape(HA, 2, 128, D)
    return np.ascontiguousarray(
        a.transpose(2, 0, 1, 3).reshape(128, HA * 2 * D))


def _host_prep(inputs):
    """Build per-core in_maps from full inputs."""
    input_ids = np.asarray(inputs["input_ids"])
    label_ids = np.asarray(inputs["label_ids"])
    emb = np.asarray(inputs["emb"], dtype=np.float32)
    anw = np.asarray(inputs["attn_norm_w"], dtype=np.float32)
    fnw = np.asarray(inputs["ffn_norm_w"], dtype=np.float32)
    finw = np.asarray(inputs["final_norm_w"], dtype=np.float32)
    lm_head = np.asarray(inputs["lm_head"], dtype=np.float32)

    # fold norm weights into the consuming matmuls
    wq = np.asarray(inputs["wq"], np.float32) * anw[:, :, None]
    wk = np.asarray(inputs["wk"], np.float32) * anw[:, :, None]
    wv = np.asarray(inputs["wv"], np.float32) * anw[:, :, None]
    wo = np.asarray(inputs["wo"], np.float32)
    w1 = np.asarray(inputs["w1"], np.float32) * fnw[:, :, None]
    w3 = np.asarray(inputs["w3"], np.float32) * fnw[:, :, None]
    w2 = np.asarray(inputs["w2"], np.float32)
    lmh_f = lm_head * finw[:, None]

    suf_ids = label_ids.reshape(-1)

    # RoPE swap permutation on the output dim: rot = W[:, sigma]
    d_i = np.arange(D)
    sigma = (d_i // HD) * HD + ((d_i % HD) + HALF) % HD
    # per-layer fp8 weight blobs
    wb = np.zeros((NL, 128, WB_SZ), dtype=FP8)
    for l in range(NL):
        wb[l, :, WB_K:WB_K + 2048] = _dr_pack(wk[l], W_SC)
        wb[l, :, WB_KR:WB_KR + 2048] = _dr_pack(wk[l][:, sigma], W_SC)
        wb[l, :, WB_Q:WB_Q + 2048] = _dr_pack(wq[l], W_SC)
        wb[l, :, WB_QR:WB_QR + 2048] = _dr_pack(wq[l][:, sigma], W_SC)
        wb[l, :, WB_V:WB_V + 2048] = _dr_pack(wv[l], W_SC)
        wb[l, :, WB_O:WB_O + 2048] = _dr_pack(wo[l], W_SC)
        wb[l, :, WB_W1:WB_W1 + 4 * HID] = _dr_pack(w1[l], W_SC)
        wb[l, :, WB_W3:WB_W3 + 4 * HID] = _dr_pack(w3[l], W_SC)
        wb[l, :, WB_W2:WB_W2 + HA * 2 * D] = _dr_pack_w2(w2[l], W2_SC)

    # RoPE tables (1/W_SC folded in; q and k each carry one factor... both
    # raw projections are x W_SC, tables carry exactly 1/W_SC)
    pos = np.concatenate(
        [np.arange(T), np.tile(T + np.arange(LBL), NLAB)]).astype(np.float32)
    freqs = 1.0 / (10000.0 ** (np.arange(HALF, dtype=np.float32) / HALF))

    def rope_tabs(positions):
        ang = positions[None, :] * freqs[:, None]
        c = np.tile(np.cos(ang), (4, 1)) / W_SC
        s32 = np.sin(ang) / W_SC
        s = np.concatenate([-s32, s32, -s32, s32], 0)
        return c, s

    ctab, stab = rope_tabs(pos)
    maskd, mq1, tq = _build_masks()
    cqt, sqt = rope_tabs(pos[tq])

    # packed trimmed mask segments (see MSEG)
    maskc = np.concatenate([
        maskd[0][:, 0:128], maskd[1][:, 128:256], maskd[2][:, QC:384],
        maskd[3][:, 384:NTOK], maskd[4][:, 512:NTOK]], axis=1)  # [128, 536]
    # thin mask duplicated for head pairs: [128, 5, 32]
    maskqc = np.ascontiguousarray(
        np.tile(mq1, (1, 1, 2)).transpose(1, 0, 2))

    sel_cols = suf_ids.astype(np.int64)
    lmsel = np.ascontiguousarray(lmh_f[:, sel_cols])       # [512, 20]
    lmsel_p = lmsel.reshape(KT, 128, NSEL).transpose(1, 0, 2)

    cbs = np.zeros((128, CB_SZ), dtype=BF16)
    cbs[:, CB_C:CB_C + NTOK] = ctab.astype(BF16)
    cbs[:, CB_S:CB_S + NTOK] = stab.astype(BF16)
    cbs[:, CB_CQ:CB_CQ + NQ] = cqt.astype(BF16)
    cbs[:, CB_SQ:CB_SQ + NQ] = sqt.astype(BF16)
    cbs[:, CB_MD:CB_MD + MSEG_SZ] = maskc.astype(BF16)
    cbs[:, CB_MQ:CB_MQ + TT * 2 * NQ] = maskqc.reshape(128, -1).astype(BF16)
    cbs[:, CB_SEL:CB_SEL + KT * NSEL] = lmsel_p.reshape(128, -1).astype(BF16)

    in_maps = []
    for c in range(NCORES):
        b = c % B
        tok = np.concatenate([input_ids[b], suf_ids])
        x0 = np.ascontiguousarray(emb[tok].T) * RS
        x0p = x0.reshape(KT, 128, NTOK).transpose(1, 0, 2).astype(BF16)
        sh = np.zeros((D, NVCH * VCH), np.float32)
        sh[:, :VSH] = lmh_f[:, c * VSH:(c + 1) * VSH] * LMH_SCALE
        lmh8 = sh.astype(FP8).reshape(2, 2, 128, NVCH, VCH)
        lmh8 = np.ascontiguousarray(lmh8.transpose(2, 3, 0, 1, 4))
        m = dict(wb=wb, cb=cbs, x0T=np.ascontiguousarray(x0p), lmh=lmh8)
        in_maps.append(m)
    return in_maps


def _host_combine(results):
    """Combine per-core partial sumexp + selected logits into [B, NLAB]."""
    se = np.zeros((NROW,), dtype=np.float64)
    for c in range(NCORES):
        # each padded column contributes exp(0)=1 to every row's partial
        se += np.asarray(results[c]["se_out"], np.float64).sum(axis=1) - VPAD
    lse = np.log(se)
    sel = np.asarray(results[0]["sel_out"], np.float64)    # [80, 20]
    rows = np.arange(NROW)
    bb = rows // (NLAB * LBL)
    ll = (rows % (NLAB * LBL)) // LBL
    jj = rows % LBL
    lp = sel[rows, ll * LBL + jj] - lse
    out = np.zeros((B, NLAB), dtype=np.float64)
    np.add.at(out, (bb, ll), lp)
    return out.astype(np.float32)


def kernel(**inputs):
    nc = _get_nc()
    in_maps = _host_prep(inputs)
    res = run_bass_kernel_spmd(
        nc, in_maps, core_ids=list(range(NCORES)),
        trace=_CACHE.get("trace", False),
    )
    _CACHE["last_results"] = res
    return _host_combine(res.results)


# revision 46
# speedup vs baseline: 1.0613x; 1.0065x over previous
"""Trainium2 Bass kernel for nn_LlamaEmbeddingClassifier.

Model: 2-layer Llama (D=512, 8 heads x 64, HID=1408, RoPE, RMSNorm) scoring
B=4 prompts against NLAB=5 label continuations (LBL=4 tokens) with an
lm_head over V=128000.

Strategy (8 NeuronCores, single SPMD launch):
  - Packed 528-token sequence [508 prefix | 5 x 4-token suffixes] with a
    custom attention mask; core c handles batch row (c % 4); cores 4-7
    duplicate 0-3 (SPMD program is uniform).  Layer 2 runs full K/V but a
    thin 16-query path for attention/MLP.
  - All weight matmuls (wq/wk/wv/wo/w1/w3/w2) run in fp8-e4m3 DoubleRow
    perf mode (2x PE throughput, half the weight HBM traffic).  The
    residual stream is stored scaled x64 so host-prescaled fp8 weights
    (x64) need no on-device compensation: RMSNorm is scale-invariant
    (eps folded as 64^2*eps), and both wo/w2 outputs land back on the
    x64 scale.  RoPE is applied via dual projections (W and W*P^T both
    in the fp8 weight blob) - no on-device transpose/permute matmul.
  - Attention probabilities are stored fp8; PV runs DoubleRow over
    key-tile pairs.  Softmax denominators come free via a ones-column in
    the value tile; exp is un-normalized (scores bounded ~|1.5|).
  - Per-layer weights are packed into one fp8 DRAM blob -> one big DMA
    per layer (512B+ contiguous runs, minimal HWDGE occupancy).  The
    fp8 lm_head shard (vocab/8 per core, padded to 16384 cols so chunk
    runs are exactly 512B) is fully resident in SBUF, prefetched during
    the transformer; the host subtracts the known exp(0) pad
    contribution from the sumexp.
  - The 16 final hidden rows per batch are AllGathered (tiny) so every
    core scores all 80 rows over its vocab shard.  Label-token logits
    come from a bf16 side matmul (lmsel) to keep the accuracy-critical
    path out of fp8.  Host combines: logsumexp across shards,
    lp = sel_logit - lse, summed per (batch,label).
"""

import math
import os
import sys
from contextlib import ExitStack

for _p in ("/opt/trn_rl_repo", "/root/.axon_site/_ro/trn_rl_repo"):
    if os.path.isdir(_p) and _p not in sys.path:
        sys.path.insert(0, _p)

import ml_dtypes
import numpy as np

import concourse.bass as bass
import concourse.tile as tile
from concourse import bacc, mybir
from concourse.bass_utils import run_bass_kernel_spmd

BF16 = ml_dtypes.bfloat16
FP8 = np.dtype(ml_dtypes.float8_e4m3)

# Problem dims (hardcoded per contract)
V, D, NH, NL, HID = 128000, 512, 8, 2, 1408
HD, HALF = 64, 32
B, T, NLAB, LBL = 4, 508, 5, 4
EPS = 1e-5
NCORES = 8
SUF = NLAB * LBL            # 20 suffix tokens
NTOK = T + SUF              # 528 packed tokens
KT = D // 128               # 4 K-tiles over D
TT = (NTOK + 127) // 128    # 5 token tiles (last has 16 rows)
TP = 3                      # key-tile pairs for DoubleRow PV
HT = HID // 128             # 11 tiles over HID
HA = 6                      # DoubleRow passes over padded HID (1536)
NROW = B * NLAB * LBL       # 80 scoring rows
NSEL = NLAB * LBL           # 20 selected lm_head columns
NQ = 1 + NLAB * (LBL - 1)   # 16 thin-path query positions
VSH = V // NCORES           # 16000 vocab shard per core
VCH = 512                   # vocab chunk (512B fp8 runs, 2KB psum bank)
NVCH = 32                   # chunks over the padded shard
VPAD = NVCH * VCH - VSH     # 384 zero-padded columns -> exp(0)=1 each
QC = 264                    # q chunk (2 chunks of 264 = 528)

RS = 64.0                   # residual-stream scale (h stores 64*h_true)
W_SC = 64.0                 # host scale for wq/wk/wv/wo/w1/w3 (fp8 range)
W2_SC = 16.0                # host scale for w2
G1_SC = 4.0                 # on-device scale of stored g1 (W2_SC*G1_SC=RS)
EPS_S = EPS * RS * RS       # rms eps on the scaled residual
LMH_SCALE = 32.0            # host premultiply of fp8 lm_head
HS_SCALE = 4.0              # device premultiply of fp8 hs copies

# causal block structure: q-chunk 0 (q<264) only sees k-tiles 0,1; q-chunk 1
# sees all.  (Queries 256..263 lose keys 256..263 - tiny, within tolerance.)
CH_MTS = {0: (0, 1), 1: (0, 1, 2, 3, 4)}
CH_TPS = {0: (0,), 1: (0, 1, 2)}   # same structure as key-tile pairs
# exp only over columns that are not fully causally masked
EXPR = {(0, 0): (0, QC), (1, 0): (128, QC),
        (0, 1): (QC, NTOK), (1, 1): (QC, NTOK), (2, 1): (QC, NTOK),
        (3, 1): (384, NTOK), (4, 1): (512, NTOK)}
# mask-multiply ranges (within the exp'd region) + offset into packed maskc
MSEG = {0: (0, 128, 0), 1: (128, 256, 128), 2: (QC, 384, 256),
        3: (384, NTOK, 376), 4: (512, NTOK, 520)}
MSEG_SZ = 536

# fp8 weight-blob element offsets (per partition), layer-major
WB_K, WB_KR, WB_Q, WB_QR = 0, 2048, 4096, 6144
WB_V, WB_O = 8192, 10240
WB_W1, WB_W3, WB_W2 = 12288, 17920, 23552
WB_SZ = 29696
# consts blob (bf16) element offsets
CB_C, CB_S = 0, NTOK
CB_CQ, CB_SQ = 2 * NTOK, 2 * NTOK + NQ
CB_MD = 2 * NTOK + 2 * NQ                 # packed trimmed mask segments
CB_MQ = CB_MD + MSEG_SZ                   # [5, 32] head-pair thin mask
CB_SEL = CB_MQ + TT * 2 * NQ              # [4, 20] lmsel
CB_SZ = CB_SEL + KT * NSEL

_CACHE = {}


def _tok_rows(tt):
    return min(128, NTOK - tt * 128)


def build_nc(use_collective=True):
    nc = bacc.Bacc("TRN2", num_devices=NCORES)
    f32, bf16, fp8 = mybir.dt.float32, mybir.dt.bfloat16, mybir.dt.float8e4
    DR = mybir.MatmulPerfMode.DoubleRow

    # ---- I/O ----
    x0T = nc.dram_tensor("x0T", [128, KT, NTOK], bf16,
                         kind="ExternalInput")
    wb = nc.dram_tensor("wb", [NL, 128, WB_SZ], fp8, kind="ExternalInput")
    cb = nc.dram_tensor("cb", [128, CB_SZ], bf16, kind="ExternalInput")
    lmh = nc.dram_tensor("lmh", [128, NVCH, 2, 2, VCH], fp8,
                         kind="ExternalInput")

    se_out = nc.dram_tensor("se_out", [NROW, NVCH], bf16,
                            kind="ExternalOutput")
    sel_out = nc.dram_tensor("sel_out", [NROW, NSEL], f32,
                             kind="ExternalOutput")

    with tile.TileContext(nc) as tc, ExitStack() as ctx:
        consts = ctx.enter_context(tc.tile_pool(name="consts", bufs=1))
        wpool = ctx.enter_context(tc.tile_pool(name="weights", bufs=1))
        lpool = ctx.enter_context(tc.tile_pool(name="lmh", bufs=1))
        acts = ctx.enter_context(tc.tile_pool(name="acts", bufs=1))
        scr = ctx.enter_context(tc.tile_pool(name="scratch", bufs=6))
        ppool = ctx.enter_context(tc.tile_pool(name="p", bufs=2))
        psum = ctx.enter_context(tc.tile_pool(name="psum", bufs=7,
                                              space="PSUM"))
        psmall = ctx.enter_context(tc.tile_pool(name="psmall", bufs=1,
                                                space="PSUM"))
        dram = ctx.enter_context(tc.tile_pool(name="dram", bufs=1,
                                              space="DRAM"))

        # ---- resident tiles ----
        h = acts.tile([128, KT, NTOK], bf16, name="h")
        cbs = consts.tile([128, CB_SZ], bf16, name="cbs")
        wsb = wpool.tile([128, NL, WB_SZ], fp8, name="wsb")
        lsb = lpool.tile([128, NVCH, 2, 2, VCH], fp8, name="lsb")

        # ---- input DMAs, in pipeline order (single shared DMA pipe) ----
        nc.sync.dma_start(out=h[:], in_=x0T[:])
        nc.sync.dma_start(out=cbs[:], in_=cb[:])
        nc.sync.dma_start(out=wsb[:, 0, :WB_V], in_=wb[0][:, :WB_V])
        nc.sync.dma_start(out=wsb[:, 0, WB_V:], in_=wb[0][:, WB_V:])
        nc.sync.dma_start(out=wsb[:, 1, :], in_=wb[1][:])
        nc.sync.dma_start(out=lsb[:, :NVCH // 2], in_=lmh[:, :NVCH // 2])
        nc.sync.dma_start(out=lsb[:, NVCH // 2:], in_=lmh[:, NVCH // 2:])

        # ---- const views ----
        C128 = cbs[:, CB_C:CB_C + NTOK]
        S128 = cbs[:, CB_S:CB_S + NTOK]
        Cq = cbs[:, CB_CQ:CB_CQ + NQ]
        Sq = cbs[:, CB_SQ:CB_SQ + NQ]
        maskc = cbs[:, CB_MD:CB_MD + MSEG_SZ]
        maskq = cbs[:, CB_MQ:CB_MQ + TT * 2 * NQ].rearrange(
            "p (t q) -> p t q", t=TT)
        lmsel_sb = cbs[:, CB_SEL:CB_SEL + KT * NSEL].rearrange(
            "p (k c) -> p k c", k=KT)

        def wv_(l, off, n, a=2):
            return wsb[:, l, off:off + a * 2 * n].rearrange(
                "p (a s n) -> p a s n", a=a, s=2)

        ones_col = consts.tile([128, 1], bf16)
        nc.vector.memset(ones_col, 1.0)
        eps_sb = consts.tile([1, 1], f32)
        nc.vector.memset(eps_sb, float(EPS_S))

        # activations
        xn8 = acts.tile([128, 2, 2, NTOK], fp8, name="xn8")
        xn8b = acts.tile([128, 2, 2, NTOK], fp8, name="xn8b")
        kTt = acts.tile([128, KT, NTOK], bf16, name="kTt")
        qT = acts.tile([128, KT, NTOK], bf16, name="qT")
        oT8 = acts.tile([128, 2, 2, NTOK], fp8, name="oT8")
        g1 = acts.tile([128, HA, 2, NTOK], fp8, name="g1")
        # v with interleaved ones column plus one pad column (so the
        # DoubleRow lhsT pair-stride is 528 = a multiple of 16 bytes), in
        # key-tile-pair layout: [keys(128), pair, slot, 8 x (64 v | one |pad)]
        HDP = HD + 2
        v_aug = acts.tile([128, TP, 2, NH * HDP], fp8, name="v_aug")
        v5 = v_aug.rearrange("p t s (h c) -> p t s h c", c=HDP)
        nc.gpsimd.memset(v5[:, :, :, :, HD:], 1.0)    # ones (+pad) columns
        nc.gpsimd.memset(v5[:, 2, 1, :, :HD], 0.0)    # pad slot (tile 5)
        nc.gpsimd.memset(v5[:, 2, 0, :, :HD], 0.0)    # tile-4 (rows 0:16 get
        # overwritten by the real v projection; tail rows stay zero)
        nc.gpsimd.memset(g1[:, HA - 1, 1, :], 0.0)    # padded HID slot

        hq = acts.tile([128, KT, NQ], bf16, name="hq")
        xnq8 = acts.tile([128, 2, 2, NQ], fp8, name="xnq8")
        xnq8b = acts.tile([128, 2, 2, NQ], fp8, name="xnq8b")
        qTq = acts.tile([128, KT, NQ], bf16, name="qTq")
        oTq8 = acts.tile([128, 2, 2, NQ], fp8, name="oTq8")
        g1q = acts.tile([128, HA, 2, NQ], fp8, name="g1q")
        nc.gpsimd.memset(g1q[:, HA - 1, 1, :], 0.0)

        FULL_CH = ((0, QC), (QC, NTOK))
        THIN_CH = ((0, NQ),)

        def rms(src, dest, n, chunks, sq_eng, mul_eng=None):
            mul_eng = mul_eng or nc.vector
            """dest (fp8, DR layout) = src * rsqrt(mean_D(src^2)+eps').

            Emitted per column-chunk so downstream projections of chunk 0
            can start while chunk 1 is still normalizing.
            """
            for c0, c1 in chunks:
                w = c1 - c0
                sq = scr.tile([128, KT, QC], bf16, name="sq", bufs=2)
                for kt in range(KT):
                    sq_eng.tensor_mul(out=sq[:, kt, :w],
                                      in0=src[:, kt, c0:c1],
                                      in1=src[:, kt, c0:c1])
                ss = psmall.tile([1, QC], f32, name="ss", tag="small")
                for kt in range(KT):
                    nc.tensor.matmul(ss[:, :w], ones_col[:], sq[:, kt, :w],
                                     start=(kt == 0), stop=(kt == KT - 1))
                # rsqrt(x) = exp(-0.5*ln(x)); Ln/Exp share an act table
                lnb = scr.tile([1, QC], f32, name="lnb")
                nc.scalar.activation(out=lnb[:, :w], in_=ss[:, :w],
                                     func=mybir.ActivationFunctionType.Ln,
                                     scale=1.0 / D, bias=eps_sb[:])
                rstd = scr.tile([1, QC], bf16, name="rstd")
                nc.scalar.activation(out=rstd[:, :w], in_=lnb[:, :w],
                                     func=mybir.ActivationFunctionType.Exp,
                                     scale=-0.5)
                rb = scr.tile([128, QC], bf16, name="rms_rb", bufs=2)
                nc.gpsimd.partition_broadcast(rb[:, :w], rstd[:, :w])
                for kt in range(KT):
                    mul_eng.tensor_mul(out=dest[:, kt // 2, kt % 2, c0:c1],
                                       in0=src[:, kt, c0:c1],
                                       in1=rb[:, :w])

        def proj_rope(specs, xn, chunks, ctb, stb, mts=None):
            """dest[Dout, n] (bf16) = rope((xn @ W).T) via two fp8 DR projs.

            specs: list of (dest, w_raw_view, w_rot_view); tiles of the
            different projections are interleaved for engine overlap.
            """
            for c0, c1 in chunks:
                n = c1 - c0
                for mt in (range(KT) if mts is None else mts):
                    msl = slice(mt * 128, (mt + 1) * 128)
                    for dest, wv_raw, wv_rot in specs:
                        psA = psum.tile([128, QC], f32, name="pjA", tag="mm")
                        psB = psum.tile([128, QC], f32, name="pjB", tag="mm")
                        for a in range(2):
                            nc.tensor.matmul(psA[:, :n], wv_raw[:, a, :, msl],
                                             xn[:, a, :, c0:c1],
                                             start=(a == 0), stop=(a == 1),
                                             perf_mode=DR)
                        for a in range(2):
                            nc.tensor.matmul(psB[:, :n], wv_rot[:, a, :, msl],
                                             xn[:, a, :, c0:c1],
                                             start=(a == 0), stop=(a == 1),
                                             perf_mode=DR)
                        t1 = scr.tile([128, QC], bf16, name="rope_t1")
                        nc.vector.tensor_mul(out=t1[:, :n], in0=psA[:, :n],
                                             in1=ctb[:, c0:c1])
                        t2 = scr.tile([128, QC], bf16, name="rope_t2")
                        nc.vector.tensor_mul(out=t2[:, :n], in0=psB[:, :n],
                                             in1=stb[:, c0:c1])
                        nc.gpsimd.tensor_add(out=dest[:, mt, c0:c1],
                                             in0=t1[:, :n], in1=t2[:, :n])

        def v_proj(wv_v, xn, n_tiles, ncols):
            """v_aug[:, tp, s, h, :HD] = (xn @ Wv).T / RS in pair layout."""
            for mt in range(n_tiles):
                mr = min(128, ncols - mt * 128)
                ps = psum.tile([128, D], f32, name="v_ps", tag="mm")
                if mr == 128:
                    for a in range(2):
                        nc.tensor.matmul(
                            ps[:mr, :], xn[:, a, :, mt * 128:mt * 128 + mr],
                            wv_v[:, a],
                            start=(a == 0), stop=(a == 1), perf_mode=DR)
                else:  # tail: plain fp8 (DR needs a full 128-col stationary)
                    for i, (a, s_) in enumerate(
                            [(a, s_) for a in range(2) for s_ in range(2)]):
                        nc.tensor.matmul(
                            ps[:mr, :], xn[:, a, s_, mt * 128:mt * 128 + mr],
                            wv_v[:, a, s_],
                            start=(i == 0), stop=(i == 3))
                nc.vector.tensor_scalar_mul(
                    out=v5[:mr, mt // 2, mt % 2, :, :HD],
                    in0=ps.rearrange("p (h c) -> p h c", c=HD)[:mr],
                    scalar1=1.0 / RS)

        def attn_norm(po, a, s, b, dest8, cs, n):
            """dest8 = po[:HD]/po[HD] (softmax denominator), fp8 out."""
            rs_t = scr.tile([1, QC], f32, name="attn_rs")
            nc.vector.reciprocal(out=rs_t[:, :n], in_=po[HD:HD + 1, :n])
            rb_t = scr.tile([64, QC], f32, name="attn_rb")
            nc.gpsimd.partition_broadcast(rb_t[:, :n], rs_t[:, :n])
            nc.vector.tensor_mul(
                out=dest8[64 * b:64 * b + 64, a, s, cs],
                in0=po[:HD, :n], in1=rb_t[:, :n])

        def attn_head_full(hh):
                tq = hh // 2
                rq = slice(64 * (hh % 2), 64 * (hh % 2) + 64)
                a, s, bb = hh // 4, (hh // 2) % 2, hh % 2
                p_sb = ppool.tile([128, TP, 2, NTOK], fp8, name="p_sb")
                if hh < 2:  # zero exp-trimmed regions in both buffers
                    nc.gpsimd.memset(p_sb[:, 0, 1, 0:128], 0.0)
                    nc.gpsimd.memset(p_sb[:, 1, 1, QC:384], 0.0)
                    nc.gpsimd.memset(p_sb[:, 2, 0, QC:], 0.0)
                    nc.gpsimd.memset(p_sb[:, 2, 1, QC:], 0.0)
                for mt in range(TT):
                    mr = _tok_rows(mt)
                    for ch in range(2):
                        if mt not in CH_MTS[ch]:
                            continue
                        e0, e1 = EXPR[(mt, ch)]
                        cs = slice(ch * QC, (ch + 1) * QC)
                        ps = psum.tile([128, QC], f32, name="score_ps",
                                       tag="mm")
                        nc.tensor.matmul(
                            ps[:mr, :e1 - e0],
                            kTt[rq, tq, mt * 128:mt * 128 + mr],
                            qT[rq, tq, e0:e1], start=True, stop=True)
                        nc.scalar.activation(
                            out=p_sb[:mr, mt // 2, mt % 2, e0:e1],
                            in_=ps[:mr, :e1 - e0],
                            func=mybir.ActivationFunctionType.Exp,
                            scale=1.0 / math.sqrt(HD))
                    m0, m1, mo = MSEG[mt]
                    nc.gpsimd.tensor_mul(
                        out=p_sb[:mr, mt // 2, mt % 2, m0:m1],
                        in0=p_sb[:mr, mt // 2, mt % 2, m0:m1],
                        in1=maskc[:mr, mo:mo + m1 - m0])
                for ch in range(2):
                    cs = slice(ch * QC, (ch + 1) * QC)
                    tps = CH_TPS[ch]
                    po = psum.tile([128, QC], f32, name="pv_ps", tag="mm")
                    for i, tp in enumerate(tps):
                        nc.tensor.matmul(
                            po[:HD + 2, :],
                            v_aug[:, tp, :, hh * HDP:(hh + 1) * HDP],
                            p_sb[:, tp, :, cs],
                            start=(i == 0), stop=(i == len(tps) - 1),
                            perf_mode=DR)
                    attn_norm(po, a, s, bb, oT8, cs, QC)

        def build_qz():
            # head-pair batching: kTt's 128 partitions hold two heads; the
            # query block is zero-padded so one matmul yields both heads'
            # scores side by side ([mr, 32], offset 0 - HW-safe).
            qz = scr.tile([128, KT, 2 * NQ], bf16, name="qz", bufs=1)
            nc.vector.memset(qz[:], 0.0)
            for tq in range(KT):
                nc.vector.tensor_copy(out=qz[0:64, tq, 0:NQ],
                                      in_=qTq[0:64, tq, :])
                nc.vector.tensor_copy(out=qz[64:128, tq, NQ:2 * NQ],
                                      in_=qTq[64:128, tq, :])
            return qz

        def attn_group_thin(tq, qz):
                p_sb = ppool.tile([128, TP, 2, 2 * NQ], fp8, name="pq_sb")
                if tq < 2:
                    nc.gpsimd.memset(p_sb[:, 2, 1, :], 0.0)
                    nc.gpsimd.memset(p_sb[:, 2, 0, :], 0.0)
                for mt in range(TT):
                    mr = _tok_rows(mt)
                    ps = psum.tile([128, QC], f32, name="score_ps", tag="mm")
                    nc.tensor.matmul(
                        ps[:mr, :2 * NQ],
                        kTt[:, tq, mt * 128:mt * 128 + mr],
                        qz[:, tq, :], start=True, stop=True)
                    nc.scalar.activation(
                        out=p_sb[:mr, mt // 2, mt % 2, :],
                        in_=ps[:mr, :2 * NQ],
                        func=mybir.ActivationFunctionType.Exp,
                        scale=1.0 / math.sqrt(HD))
                    nc.gpsimd.tensor_mul(
                        out=p_sb[:mr, mt // 2, mt % 2, :],
                        in0=p_sb[:mr, mt // 2, mt % 2, :],
                        in1=maskq[:mr, mt, :])
                for half in range(2):
                    hh = 2 * tq + half
                    a, s, bb = hh // 4, (hh // 2) % 2, hh % 2
                    nsl = slice(half * NQ, (half + 1) * NQ)
                    po = psum.tile([128, QC], f32, name="pv_ps", tag="mm")
                    for tp in range(TP):
                        nc.tensor.matmul(
                            po[:HD + 2, :NQ],
                            v_aug[:, tp, :, hh * HDP:(hh + 1) * HDP],
                            p_sb[:, tp, :, nsl],
                            start=(tp == 0), stop=(tp == TP - 1),
                            perf_mode=DR)
                    attn_norm(po, a, s, bb, oTq8, slice(0, NQ), NQ)

        def accum_proj_dr(w_view, npass, src8, dest, chunks, add_eng):
            """dest += (src8 DR-matmul w).T ; w_view [128, npass, 2, Dout]."""
            for c0, c1 in chunks:
                n = c1 - c0
                for mt in range(KT):
                    msl = slice(mt * 128, (mt + 1) * 128)
                    ps = psum.tile([128, QC], f32, name="acc_ps", tag="mm")
                    for a in range(npass):
                        nc.tensor.matmul(
                            ps[:, :n], w_view[:, a, :, msl],
                            src8[:, a, :, c0:c1],
                            start=(a == 0), stop=(a == npass - 1),
                            perf_mode=DR)
                    add_eng.tensor_add(out=dest[:, mt, c0:c1],
                                       in0=dest[:, mt, c0:c1], in1=ps[:, :n])

        def mlp(l, xn, gdest, chunks, dest, n_ht, add_eng):
            w1v, w3v = wv_(l, WB_W1, HID), wv_(l, WB_W3, HID)
            w2v = wv_(l, WB_W2, D, a=HA)
            for ci, (c0, c1) in enumerate(chunks):
                n = c1 - c0
                for mt in range(n_ht):
                    msl = slice(mt * 128, (mt + 1) * 128)
                    ps3 = psum.tile([128, QC], f32, name="g3_ps", tag="mm")
                    for a in range(2):
                        nc.tensor.matmul(ps3[:, :n], w3v[:, a, :, msl],
                                         xn[:, a, :, c0:c1],
                                         start=(a == 0), stop=(a == 1),
                                         perf_mode=DR)
                    ps1 = psum.tile([128, QC], f32, name="g1_ps", tag="mm")
                    for a in range(2):
                        nc.tensor.matmul(ps1[:, :n], w1v[:, a, :, msl],
                                         xn[:, a, :, c0:c1],
                                         start=(a == 0), stop=(a == 1),
                                         perf_mode=DR)
                    tsil = scr.tile([128, QC], bf16, name="tsil")
                    nc.scalar.activation(
                        out=tsil[:, :n], in_=ps1[:, :n],
                        func=mybir.ActivationFunctionType.Silu,
                        scale=1.0 / W_SC)
                    # g1 = (ps3 * G1_SC/W_SC) * silu  (fused; DVE - reads PSUM)
                    nc.vector.scalar_tensor_tensor(
                        out=gdest[:, mt // 2, mt % 2, c0:c1],
                        in0=ps3[:, :n], scalar=G1_SC / W_SC,
                        in1=tsil[:, :n],
                        op0=mybir.AluOpType.mult, op1=mybir.AluOpType.mult)
            accum_proj_dr(w2v, HA, gdest, dest, chunks, add_eng)

        def gather_q(dest, src):
            """cols: 0 <- 507; 1+3l+j <- 508+4l+j (j=0..2); [128, k, cols]"""
            for kt in range(src.shape[1]):
                nc.vector.tensor_copy(out=dest[:, kt, 0:1],
                                      in_=src[:, kt, T - 1:T])
                nc.vector.tensor_copy(
                    out=dest[:, kt, 1:NQ].rearrange("p (l s) -> p l s", s=3),
                    in_=src[:, kt, T:T + SUF].rearrange(
                        "p (l s) -> p l s", s=LBL)[:, :, 0:3])

        def gather_q4(dest, src):
            for a in range(2):
                for s_ in range(2):
                    nc.vector.tensor_copy(out=dest[:, a, s_, 0:1],
                                          in_=src[:, a, s_, T - 1:T])
                    nc.vector.tensor_copy(
                        out=dest[:, a, s_, 1:NQ].rearrange(
                            "p (l s) -> p l s", s=3),
                        in_=src[:, a, s_, T:T + SUF].rearrange(
                            "p (l s) -> p l s", s=LBL)[:, :, 0:3])

        # ================= transformer =================
        # Attention of heads (2t, 2t+1) only needs k/q tile t, so emit each
        # projection tile followed immediately by its two heads - the PE's
        # in-order queue + psum backpressure otherwise delays the first
        # score matmul until every projection tile has drained.
        for l in range(NL):
            full = l < NL - 1
            rms(h, xn8, NTOK, FULL_CH,
                nc.vector if full else nc.gpsimd, nc.vector)
            if full:
                specs = [(kTt, wv_(l, WB_K, D), wv_(l, WB_KR, D)),
                         (qT, wv_(l, WB_Q, D), wv_(l, WB_QR, D))]
                proj_rope(specs, xn8, FULL_CH, C128, S128)
                v_proj(wv_(l, WB_V, D), xn8, TT, NTOK)
                for hh in range(NH):
                    attn_head_full(hh)
                accum_proj_dr(wv_(l, WB_O, D), 2, oT8, h, FULL_CH, nc.vector)
                rms(h, xn8b, NTOK, FULL_CH, nc.gpsimd, nc.gpsimd)
                mlp(l, xn8b, g1, FULL_CH, h, HT, nc.vector)
            else:
                gather_q(hq, h)
                gather_q4(xnq8, xn8)
                proj_rope([(qTq, wv_(l, WB_Q, D), wv_(l, WB_QR, D))],
                          xnq8, THIN_CH, Cq, Sq)
                qz = build_qz()
                kspec = [(kTt, wv_(l, WB_K, D), wv_(l, WB_KR, D))]
                proj_rope(kspec, xn8, FULL_CH, C128, S128)
                v_proj(wv_(l, WB_V, D), xn8, TT, NTOK)
                for tq in range(KT):
                    attn_group_thin(tq, qz)
                accum_proj_dr(wv_(l, WB_O, D), 2, oTq8, hq, THIN_CH,
                              nc.vector)
                rms(hq, xnq8b, NQ, THIN_CH, nc.vector)
                mlp(l, xnq8b, g1q, THIN_CH, hq, HT, nc.vector)

        # ============ final norm + extract + AllGather ============
        xnf = scr.tile([128, KT, NQ], bf16, name="xnf")
        # bf16 out (not fp8): feeds the accuracy-critical lmsel path
        sqf = scr.tile([128, KT, NQ], bf16, name="sqf", bufs=1)
        for kt in range(KT):
            nc.vector.tensor_mul(out=sqf[:, kt, :], in0=hq[:, kt, :],
                                 in1=hq[:, kt, :])
        ssf = psmall.tile([1, QC], f32, name="ss", tag="small")
        for kt in range(KT):
            nc.tensor.matmul(ssf[:, :NQ], ones_col[:], sqf[:, kt, :],
                             start=(kt == 0), stop=(kt == KT - 1))
        lnf = scr.tile([1, QC], f32, name="lnb")
        nc.scalar.activation(out=lnf[:, :NQ], in_=ssf[:, :NQ],
                             func=mybir.ActivationFunctionType.Ln,
                             scale=1.0 / D, bias=eps_sb[:])
        rstdf = scr.tile([1, NQ], bf16, name="rstdf")
        nc.scalar.activation(out=rstdf[:], in_=lnf[:, :NQ],
                             func=mybir.ActivationFunctionType.Exp,
                             scale=-0.5)
        rbf = scr.tile([128, NQ], bf16, name="rbf")
        nc.gpsimd.partition_broadcast(rbf[:], rstdf[:])
        for kt in range(KT):
            nc.vector.tensor_mul(out=xnf[:, kt, :], in0=hq[:, kt, :],
                                 in1=rbf[:])

        hsT_own = acts.tile([128, KT, NSEL], bf16, name="hsT_own")
        for kt in range(KT):
            eng = nc.vector if kt % 2 == 0 else nc.gpsimd
            for ll in range(NLAB):
                eng.tensor_copy(
                    out=hsT_own[:, kt, ll * LBL:ll * LBL + 1],
                    in_=xnf[:, kt, 0:1])
            eng.tensor_copy(
                out=hsT_own.rearrange("p k (l s) -> p k l s", s=LBL)[
                    :, kt, :, 1:LBL],
                in_=xnf[:, kt, 1:NQ].rearrange("p (l s) -> p l s", s=3))

        cc_in = dram.tile([D, NSEL], bf16)
        cc_out = dram.tile([NCORES * D, NSEL], bf16)
        nc.sync.dma_start(
            out=cc_in.rearrange("(k p) c -> p k c", p=128), in_=hsT_own[:])
        if use_collective:
            nc.gpsimd.collective_compute(
                "AllGather",
                mybir.AluOpType.bypass,
                replica_groups=[list(range(NCORES))],
                ins=[cc_in.opt()],
                outs=[cc_out.opt()],
            )
        else:  # timeline-sim variant: emulate with local copies, spread
            # across four queues so the copies run concurrently
            qs = [nc.sync, nc.scalar, nc.gpsimd]
            for r in range(NCORES):
                qs[r % 3].dma_start(
                    out=cc_out[r * D:(r + 1) * D, :], in_=cc_in[:])

        hsT_all = acts.tile([128, KT, B, NSEL], bf16, name="hsT_all")
        cc_view = cc_out.rearrange("(b k p) c -> p k b c", b=NCORES, p=128)
        qs2 = [nc.sync, nc.scalar, nc.gpsimd, nc.scalar]
        for kt in range(KT):
            qs2[kt].dma_start(out=hsT_all[:, kt], in_=cc_view[:, kt, 0:B, :])

        # ================= lm_head phase =================
        hs8 = acts.tile([128, 2, 2, B * NSEL], fp8, name="hs8")
        for kt in range(KT):
            nc.vector.tensor_scalar_mul(
                out=hs8[:, kt // 2, kt % 2, :],
                in0=hsT_all.rearrange("p k b c -> p k (b c)")[:, kt, :],
                scalar1=HS_SCALE)
        se_sb = acts.tile([NROW, NVCH], bf16, name="se_sb")
        escale = 1.0 / (LMH_SCALE * HS_SCALE)
        for j in range(NVCH):
            pl = psum.tile([NROW, VCH], f32, name="lm_ps", tag="mm")
            for a in range(2):
                nc.tensor.matmul(pl[:], hs8[:, a], lsb[:, j, a],
                                 start=(a == 0), stop=(a == 1),
                                 perf_mode=DR)
            esc = scr.tile([NROW, VCH], bf16, name="esc", bufs=3)
            nc.scalar.activation(
                out=esc[:], in_=pl[:],
                func=mybir.ActivationFunctionType.Exp,
                scale=escale)
            # row-sum on the (otherwise idle) DVE instead of the scalar
            # engine's accumulator read
            with nc.allow_low_precision("bf16 partial sumexp: one rounding "
                                        "per 512-col chunk, ~0.07% on lse"):
                nc.vector.tensor_reduce(out=se_sb[:, j:j + 1], in_=esc[:],
                                        axis=mybir.AxisListType.X,
                                        op=mybir.AluOpType.add)
        nc.sync.dma_start(out=se_out[:], in_=se_sb[:])

        psel = psmall.tile([NROW, NSEL], f32, name="sel_ps", tag="small")
        for kt in range(KT):
            nc.tensor.matmul(
                psel[:],
                hsT_all.rearrange("p k b c -> p k (b c)")[:, kt, :],
                lmsel_sb[:, kt, :],
                start=(kt == 0), stop=(kt == KT - 1))
        sel_sb = scr.tile([NROW, NSEL], f32, name="sel_sb")
        nc.scalar.copy(out=sel_sb[:], in_=psel[:])
        nc.sync.dma_start(out=sel_out[:], in_=sel_sb[:])

    nc.finalize()
    return nc


def _get_nc():
    if "nc" not in _CACHE:
        _CACHE["nc"] = build_nc()
    return _CACHE["nc"]


def _build_masks():
    """full mask [TT,128,NTOK] and thin mask [TT,128,NQ] over (k, q)."""
    k_idx = np.arange(TT * 128)
    kpos = np.where(k_idx < T, k_idx, 0)
    klab = np.where(k_idx < T, -1, (k_idx - T) // LBL)
    koff = np.where(k_idx < T, 0, (k_idx - T) % LBL)
    kvalid = k_idx < NTOK

    def allow(qpos, qlab, qoff):
        kp = kpos[:, None]; kl = klab[:, None]; ko = koff[:, None]
        prefix_k = kl == -1
        prefix_q = (qlab == -1)[None, :]
        a = np.where(
            prefix_q,
            prefix_k & (kp <= qpos[None, :]),
            prefix_k | ((kl == qlab[None, :]) & (ko <= qoff[None, :])),
        )
        return (a & kvalid[:, None]).astype(np.float32)

    q_idx = np.arange(NTOK)
    qpos = np.where(q_idx < T, q_idx, 0)
    qlab = np.where(q_idx < T, -1, (q_idx - T) // LBL)
    qoff = np.where(q_idx < T, 0, (q_idx - T) % LBL)
    maskd = allow(qpos, qlab, qoff).reshape(TT, 128, NTOK)

    tq = np.array([T - 1] + [T + 4 * l + j for l in range(NLAB)
                             for j in range(3)])
    qpos = np.where(tq < T, tq, 0)
    qlab = np.where(tq < T, -1, (tq - T) // LBL)
    qoff = np.where(tq < T, 0, (tq - T) % LBL)
    mq1 = allow(qpos, qlab, qoff).reshape(TT, 128, NQ)
    return maskd, mq1, tq


def _dr_pack(w, scale):
    """[512, N] f32 -> [128, 2*2*N] fp8 flat (a, s, N) DoubleRow layout."""
    N = w.shape[1]
    a = (w * scale).astype(FP8).reshape(2, 2, 128, N)
    return np.ascontiguousarray(
        a.transpose(2, 0, 1, 3).reshape(128, 4 * N))


def _dr_pack_w2(w, scale):
    """[1408, 512] f32 -> [128, 6*2*512] fp8 flat, HID padded to 1536."""
    wp = np.zeros((HA * 256, D), np.float32)
    wp[:HID] = w
    a = (wp * scale).astype(FP8).reshape(HA, 2, 128, D)
    return np.ascontiguousarray(
        a.transpose(2, 0, 1, 3).reshape(128, HA * 2 * D))


def _host_prep(inputs):
    """Build per-core in_maps from full inputs."""
    input_ids = np.asarray(inputs["input_ids"])
    label_ids = np.asarray(inputs["label_ids"])
    emb = np.asarray(inputs["emb"], dtype=np.float32)
    anw = np.asarray(inputs["attn_norm_w"], dtype=np.float32)
    fnw = np.asarray(inputs["ffn_norm_w"], dtype=np.float32)
    finw = np.asarray(inputs["final_norm_w"], dtype=np.float32)
    lm_head = np.asarray(inputs["lm_head"], dtype=np.float32)

    # fold norm weights into the consuming matmuls
    wq = np.asarray(inputs["wq"], np.float32) * anw[:, :, None]
    wk = np.asarray(inputs["wk"], np.float32) * anw[:, :, None]
    wv = np.asarray(inputs["wv"], np.float32) * anw[:, :, None]
    wo = np.asarray(inputs["wo"], np.float32)
    w1 = np.asarray(inputs["w1"], np.float32) * fnw[:, :, None]
    w3 = np.asarray(inputs["w3"], np.float32) * fnw[:, :, None]
    w2 = np.asarray(inputs["w2"], np.float32)
    lmh_f = lm_head * finw[:, None]

    suf_ids = label_ids.reshape(-1)

    # RoPE swap permutation on the output dim: rot = W[:, sigma]
    d_i = np.arange(D)
    sigma = (d_i // HD) * HD + ((d_i % HD) + HALF) % HD
    # per-layer fp8 weight blobs
    wb = np.zeros((NL, 128, WB_SZ), dtype=FP8)
    for l in range(NL):
        wb[l, :, WB_K:WB_K + 2048] = _dr_pack(wk[l], W_SC)
        wb[l, :, WB_KR:WB_KR + 2048] = _dr_pack(wk[l][:, sigma], W_SC)
        wb[l, :, WB_Q:WB_Q + 2048] = _dr_pack(wq[l], W_SC)
        wb[l, :, WB_QR:WB_QR + 2048] = _dr_pack(wq[l][:, sigma], W_SC)
        wb[l, :, WB_V:WB_V + 2048] = _dr_pack(wv[l], W_SC)
        wb[l, :, WB_O:WB_O + 2048] = _dr_pack(wo[l], W_SC)
        wb[l, :, WB_W1:WB_W1 + 4 * HID] = _dr_pack(w1[l], W_SC)
        wb[l, :, WB_W3:WB_W3 + 4 * HID] = _dr_pack(w3[l], W_SC)
        wb[l, :, WB_W2:WB_W2 + HA * 2 * D] = _dr_pack_w2(w2[l], W2_SC)

    # RoPE tables (1/W_SC folded in; q and k each carry one factor... both
    # raw projections are x W_SC, tables carry exactly 1/W_SC)
    pos = np.concatenate(
        [np.arange(T), np.tile(T + np.arange(LBL), NLAB)]).astype(np.float32)
    freqs = 1.0 / (10000.0 ** (np.arange(HALF, dtype=np.float32) / HALF))

    def rope_tabs(positions):
        ang = positions[None, :] * freqs[:, None]
        c = np.tile(np.cos(ang), (4, 1)) / W_SC
        s32 = np.sin(ang) / W_SC
        s = np.concatenate([-s32, s32, -s32, s32], 0)
        return c, s

    ctab, stab = rope_tabs(pos)
    maskd, mq1, tq = _build_masks()
    cqt, sqt = rope_tabs(pos[tq])

    # packed trimmed mask segments (see MSEG)
    maskc = np.concatenate([
        maskd[0][:, 0:128], maskd[1][:, 128:256], maskd[2][:, QC:384],
        maskd[3][:, 384:NTOK], maskd[4][:, 512:NTOK]], axis=1)  # [128, 536]
    # thin mask duplicated for head pairs: [128, 5, 32]
    maskqc = np.ascontiguousarray(
        np.tile(mq1, (1, 1, 2)).transpose(1, 0, 2))

    sel_cols = suf_ids.astype(np.int64)
    lmsel = np.ascontiguousarray(lmh_f[:, sel_cols])       # [512, 20]
    lmsel_p = lmsel.reshape(KT, 128, NSEL).transpose(1, 0, 2)

    cbs = np.zeros((128, CB_SZ), dtype=BF16)
    cbs[:, CB_C:CB_C + NTOK] = ctab.astype(BF16)
    cbs[:, CB_S:CB_S + NTOK] = stab.astype(BF16)
    cbs[:, CB_CQ:CB_CQ + NQ] = cqt.astype(BF16)
    cbs[:, CB_SQ:CB_SQ + NQ] = sqt.astype(BF16)
    cbs[:, CB_MD:CB_MD + MSEG_SZ] = maskc.astype(BF16)
    cbs[:, CB_MQ:CB_MQ + TT * 2 * NQ] = maskqc.reshape(128, -1).astype(BF16)
    cbs[:, CB_SEL:CB_SEL + KT * NSEL] = lmsel_p.reshape(128, -1).astype(BF16)

    in_maps = []
    for c in range(NCORES):
        b = c % B
        tok = np.concatenate([input_ids[b], suf_ids])
        x0 = np.ascontiguousarray(emb[tok].T) * RS
        x0p = x0.reshape(KT, 128, NTOK).transpose(1, 0, 2).astype(BF16)
        sh = np.zeros((D, NVCH * VCH), np.float32)
        sh[:, :VSH] = lmh_f[:, c * VSH:(c + 1) * VSH] * LMH_SCALE
        lmh8 = sh.astype(FP8).reshape(2, 2, 128, NVCH, VCH)
        lmh8 = np.ascontiguousarray(lmh8.transpose(2, 3, 0, 1, 4))
        m = dict(wb=wb, cb=cbs, x0T=np.ascontiguousarray(x0p), lmh=lmh8)
        in_maps.append(m)
    return in_maps


def _host_combine(results):
    """Combine per-core partial sumexp + selected logits into [B, NLAB]."""
    se = np.zeros((NROW,), dtype=np.float64)
    for c in range(NCORES):
        # each padded column contributes exp(0)=1 to every row's partial
        se += np.asarray(results[c]["se_out"], np.float64).sum(axis=1) - VPAD
    lse = np.log(se)
    sel = np.asarray(results[0]["sel_out"], np.float64)    # [80, 20]
    rows = np.arange(NROW)
    bb = rows // (NLAB * LBL)
    ll = (rows % (NLAB * LBL)) // LBL
    jj = rows % LBL
    lp = sel[rows, ll * LBL + jj] - lse
    out = np.zeros((B, NLAB), dtype=np.float64)
    np.add.at(out, (bb, ll), lp)
    return out.astype(np.float32)


def kernel(**inputs):
    nc = _get_nc()
    in_maps = _host_prep(inputs)
    res = run_bass_kernel_spmd(
        nc, in_maps, core_ids=list(range(NCORES)),
        trace=_CACHE.get("trace", False),
    )
    _CACHE["last_results"] = res
    return _host_combine(res.results)
